# revision 1
# baseline (speedup 1.0000x reference)
"""ViT attention block (B=64, N=197, H=12, hd=64, D=768) on 8 trn2 NeuronCores.

Pure data-parallel: 8 batches per core.  Per-core pipeline (all matmuls bf16,
fp32 PSUM accumulation):

  xT  <- DMA-transpose(x bf16)                        [768, 1600]
  q,k <- W_qk @ xT   per-head M=64 tiles -> qkT[64, 24 heads, tok]
         (q pre-scaled by 1/8 on host, +q_bias folded into eviction)
  v   <- xT.T @ W_v  (natural layout [tok, feat], batch-aligned M-tiles)
  per batch b, head-pair g:
    S[m,n]   = k[:,m].T q[:,n]           (keys on partitions, base-0 operands)
    E        = exp(S) * exp_rpb[m,h,n]   (ACT exp from PSUM, DVE mul;
                                          exp(rpb) precomputed on host)
    sums     = ones[msz,128].T @ E       (PE reduction over keys, result
                                          replicated across all partitions)
    B        = 1/sums                    (DVE reciprocal_approx_fast off PSUM)
    O_h[d,n] = matmul(lhsT=v_h, rhs=E_h) (one base-0 [64,512] PSUM tile/head)
    outT     = O * B                     (DVE mul, fused normalize+evict)
  y = outT.T @ proj_wT (12 K=64 chunks) + proj_b row added during eviction
      (v_bias pre-folded into proj_b on host: softmax rows sum to 1)

Hardware constraints discovered on this trn2 revision and honored throughout:
every PE operand (lhsT/rhs) AND every matmul PSUM output must sit at
base_partition 0 (upper-quadrant streaming crashes; base-64 outputs corrupt);
one accumulation group per PSUM bank; concurrent row-group matmuls need
disjoint banks; DMA-transposes are barrier-separated from DMA copies (xbar
hang); output stores go through gpsimd SWDGE.  These force the per-head
M=64/K=64 tilings (~1.5x PE cost vs. full-width ideal).
"""

import os
import sys

import numpy as np

for _p in ("/opt/trn_rl_repo", os.path.expanduser("~/.axon_site/_ro/trn_rl_repo")):
    if os.path.isdir(_p) and _p not in sys.path:
        sys.path.insert(0, _p)

import ml_dtypes  # noqa: E402

B = 64
NTOK = 197
DIM = 768
HEADS = 12
HD = 64
NCORES = 8
BS = B // NCORES  # 8 batches per core
NT = BS * NTOK  # 1576 real tokens per core
NTP = 1600  # padded tokens (mult of 16 for xbar; 12x128 + 64)
SCALE = HD ** -0.5

F32 = None  # filled on bass import
BF16 = None

_CACHE = {}


def _build_bass(stop_after=None):
    stop_after = stop_after or os.environ.get("K_STOP_AFTER", "")
    import concourse.mybir as mybir
    import concourse.tile as tile
    from concourse import bacc

    f32 = mybir.dt.float32
    bf16 = mybir.dt.bfloat16
    EXP = mybir.ActivationFunctionType.Exp

    nc = bacc.Bacc(
        "TRN2", target_bir_lowering=False, debug=False,
        num_devices=int(os.environ.get("K_NDEV", str(NCORES))),
    )

    x_d = nc.dram_tensor("x", [NTP, DIM], bf16, kind="ExternalInput")
    qkvw_d = nc.dram_tensor("qkv_wt", [DIM, 3 * DIM], bf16, kind="ExternalInput")
    qb_d = nc.dram_tensor("qb", [12, 64, 1], f32, kind="ExternalInput")
    projw_d = nc.dram_tensor("proj_wt", [DIM, DIM], bf16, kind="ExternalInput")
    pb_d = nc.dram_tensor("pb", [128, DIM], bf16, kind="ExternalInput")
    rpb_d = nc.dram_tensor("exp_rpb", [2, 128, HEADS * NTOK], bf16, kind="ExternalInput")
    y_d = nc.dram_tensor("y", [NT, DIM], f32, kind="ExternalOutput")

    # q,k feature-tile count: 12 tiles of 128 (q: 0..5, k: 6..11)
    NQK = 12
    # token n-tiles for qk phase
    NTILES = [(0, 512), (512, 512), (1024, 512), (1536, 64)]
    VTILES = [(0, 512), (512, 256)]

    with tile.TileContext(nc, linearize=bool(os.environ.get("K_LINEARIZE"))) as tc:
        with (
            tc.tile_pool(name="consts", bufs=1) as consts,
            tc.tile_pool(name="acts", bufs=1) as acts,
        ):
            # ---- constant + input loads ----
            projw = consts.tile([64, HEADS, DIM], bf16)
            rpb = consts.tile([128, 2, HEADS * NTOK], bf16)
            qb = consts.tile([64, 12, 1], f32)
            pb = consts.tile([128, DIM], bf16)
            ones = consts.tile([128, 128], bf16)

            # persistent activations
            qkT = acts.tile([64, 2 * HEADS, NTP], bf16)  # q heads 0-11, k heads 12-23
            vsb = acts.tile([128, 2 * BS, DIM], bf16)  # v natural, per (b, chunk)

            projw_v = projw_d[:].rearrange("(k p) n -> p k n", p=64)
            for k in range(HEADS):
                nc.sync.dma_start(out=projw[:, k, :], in_=projw_v[:, k, :])
            for mc in range(2):
                nc.sync.dma_start(out=rpb[:, mc, :], in_=rpb_d[mc, :, :])
            nc.sync.dma_start(out=qb[:, :, :], in_=qb_d[:].rearrange("k p o -> p k o"))
            nc.sync.dma_start(out=pb[:, :], in_=pb_d[:, :])
            nc.vector.memset(ones[:, :], 1.0)

            do_qkv = stop_after != "load"
            do_attn = do_qkv and stop_after != "qkv"
            do_proj = do_attn and stop_after != "attn"


            # ---- qkv projections ----
            with (
                tc.tile_pool(name="ldp", bufs=1) as ldp,
                tc.tile_pool(name="ps_qk", bufs=4, space="PSUM") as ps_qk,
                tc.tile_pool(name="ps_v", bufs=2, space="PSUM") as ps_v,
            ):
                qkvw = ldp.tile([128, 6, 3 * DIM], bf16)
                xt = ldp.tile([128, 6, NTP], bf16)  # x transposed [c, tok]
                for k in range(6):
                    nc.sync.dma_start_transpose(
                        out=xt[:, k, :], in_=x_d[:, k * 128 : (k + 1) * 128]
                    )
                tc.strict_bb_all_engine_barrier()
                qkvw_v = qkvw_d[:].rearrange("(k p) n -> p k n", p=128)
                for k in range(6):
                    nc.sync.dma_start(out=qkvw[:, k, :], in_=qkvw_v[:, k, :])
                for m in range(min(2 * HEADS, int(os.environ.get("K_NQKM", "99"))) if do_qkv else 0):
                    for noff, nsz in NTILES:
                        ps = ps_qk.tile([64, 512], f32)
                        for k in range(6):
                            nc.tensor.matmul(
                                ps[:, :nsz],
                                qkvw[:, k, m * 64 : (m + 1) * 64],
                                xt[:, k, noff : noff + nsz],
                                start=(k == 0),
                                stop=(k == 5),
                            )
                        if m < HEADS:  # q: add bias (pre-scaled on host)
                            nc.vector.tensor_scalar_add(
                                qkT[:, m, noff : noff + nsz],
                                ps[:, :nsz],
                                qb[:, m, 0:1],
                            )
                        else:  # k: plain copy
                            nc.scalar.copy(qkT[:, m, noff : noff + nsz], ps[:, :nsz])

                for b in range(min(BS, int(os.environ.get("K_NVB", "99"))) if do_qkv else 0):
                    for mc in range(2):
                        msz = 128 if mc == 0 else NTOK - 128
                        toff = b * NTOK + mc * 128
                        psv = ps_v.tile([128, DIM], f32)
                        for k in range(6):
                            for noff, nsz in VTILES:
                                nc.tensor.matmul(
                                    psv[:msz, noff : noff + nsz],
                                    xt[:, k, toff : toff + msz],
                                    qkvw[:, k, 1536 + noff : 1536 + noff + nsz],
                                    start=(k == 0),
                                    stop=(k == 5),
                                )
                        nc.scalar.copy(vsb[:msz, b * 2 + mc, :], psv[:msz, :])

            otp_cm = tc.tile_pool(name="otp", bufs=1)
            otp = otp_cm.__enter__()
            outT = otp.tile([64, HEADS, NTP], bf16)  # attn out, per head
            nc.vector.memset(outT[:, :, :], 0.0)

            if stop_after == "qkv":
                nc.gpsimd.dma_start(out=y_d[0:64, :], in_=qkT[:, 0, 0:DIM])
                nc.gpsimd.dma_start(out=y_d[128:256, :], in_=vsb[:, 0, :])

            # ---- attention ----
            work = tc2 = None
            SUB = os.environ.get("K_ATTN_SUB", "all")
            sublv = {"scores":0, "exp":1, "mul":2, "sums":3, "recip":4, "av":5, "final":6, "all":9}[SUB]
            NB = int(os.environ.get("K_NB", str(BS)))
            NG = int(os.environ.get("K_NG", "6"))
            if do_attn:
             with (
                tc.tile_pool(name="work", bufs=3) as work,
                tc.tile_pool(name="e2p", bufs=2) as e2p,
                tc.tile_pool(name="bp", bufs=2) as bp,
                tc.tile_pool(name="ps_s", bufs=2, space="PSUM") as ps_s,
                tc.tile_pool(name="ps_sum", bufs=1, space="PSUM") as ps_sum,
                tc.tile_pool(name="ps_o", bufs=3, space="PSUM") as ps_o,
            ):
                NGG = int(os.environ.get("K_NGG", "3"))
                for b in range(NB):
                    tb = b * NTOK
                    e2 = e2p.tile([128, 2, HEADS * NTOK], bf16)
                    if os.environ.get("K_E2MEMSET"):
                        nc.vector.memset(e2[:, :, :], 0.0)
                    Bt = bp.tile([64, HEADS * NTOK], f32)
                    for g in range(NG):  # head pairs
                        sm = ps_sum.tile([128, 512], f32)
                        for mc in range(2):
                            msz = 128 if mc == 0 else NTOK - 128
                            # one psum BANK per head: concurrent matmuls on
                            # different PE row-groups must not share a bank
                            S = ps_s.tile([128, 1024], f32)
                            for hh in range(2):
                                h = 2 * g + hh
                                nc.tensor.matmul(
                                    S[:msz, hh * 512 : hh * 512 + NTOK],
                                    qkT[:, HEADS + h, tb + mc * 128 : tb + mc * 128 + msz],
                                    qkT[:, h, tb : tb + NTOK],
                                    start=True,
                                    stop=True,
                                )
                            if sublv < 1: continue
                            exps = work.tile([128, 2 * NTOK], bf16)
                            nc.scalar.activation(
                                exps[:msz, :].rearrange("p (s n) -> p s n", s=2),
                                S[:msz, :].rearrange("p (s n) -> p s n", s=2)[:, :, :NTOK],
                                EXP,
                            )
                            if sublv < 2: continue
                            nc.vector.tensor_mul(
                                e2[:msz, mc, g * 2 * NTOK : (g + 1) * 2 * NTOK],
                                exps[:msz, :],
                                rpb[:msz, mc, g * 2 * NTOK : (g + 1) * 2 * NTOK],
                            )
                            # column sums of both heads (394 contiguous cols),
                            # replicated across all 128 partitions; single
                            # accumulation group per psum bank
                            if sublv < 3: continue
                            nc.tensor.matmul(
                                sm[:, 0 : 2 * NTOK],
                                ones[:msz, :],
                                e2[:msz, mc, g * 2 * NTOK : (g + 1) * 2 * NTOK],
                                start=(mc == 0),
                                stop=(mc == 1),
                            )
                        # 1/sums, pair-structured: rows 0:64 <- even head,
                        # rows 64:128 <- odd head (sums are replicated, so any
                        # partition range of sm holds valid data)
                        for hh in range(2 if sublv >= 4 else 0):
                            nc.vector.reciprocal_approx_fast(
                                out=Bt[0:64, (2 * g + hh) * NTOK : (2 * g + hh + 1) * NTOK],
                                in_=sm[0:64, hh * NTOK : (hh + 1) * NTOK],
                            )

                    for h in range(4 * NGG if sublv >= 5 else 0):
                        O = ps_o.tile([64, 512], f32)
                        for mc in range(2):
                            msz = 128 if mc == 0 else NTOK - 128
                            nc.tensor.matmul(
                                O[0:64, 0:NTOK],
                                vsb[:msz, b * 2 + mc, h * 64 : (h + 1) * 64],
                                e2[:msz, mc, h * NTOK : (h + 1) * NTOK],
                                start=(mc == 0),
                                stop=(mc == 1),
                            )
                        if sublv >= 6:
                            nc.vector.tensor_mul(
                                outT[:, h, tb : tb + NTOK],
                                O[0:64, 0:NTOK],
                                Bt[0:64, h * NTOK : (h + 1) * NTOK],
                            )

            if os.environ.get("K_BARRIER"):
                tc.strict_bb_all_engine_barrier()
            if stop_after == "attn" and not os.environ.get("K_NOPROBE"):
                nc.gpsimd.dma_start(out=y_d[0:64, :], in_=outT[:, 0, 0:DIM])

            # ---- output projection ----
            if do_proj:
             with (
                tc.tile_pool(name="yp", bufs=2) as yp,
                tc.tile_pool(name="ps_y", bufs=2, space="PSUM") as ps_y,
             ):
                for m in range(int(os.environ.get("K_NM", "13"))):
                    moff = m * 128
                    msz = min(128, NTP - moff)
                    real = min(128, NT - moff)
                    Y = ps_y.tile([128, DIM], f32)
                    for noff, nsz in VTILES:
                        for k in range(HEADS):
                            nc.tensor.matmul(
                                Y[:msz, noff : noff + nsz],
                                outT[:, k, moff : moff + msz],
                                projw[:, k, noff : noff + nsz],
                                start=(k == 0),
                                stop=(k == HEADS - 1),
                            )

                    ysb = yp.tile([128, DIM], f32)
                    nc.vector.tensor_add(ysb[:msz, :], Y[:msz, :], pb[:msz, :])
                    if not os.environ.get("K_NOYDMA"):
                        nc.gpsimd.dma_start(out=y_d[moff : moff + real, :], in_=ysb[:real, :])
            otp_cm.__exit__(None, None, None)

    nc.compile()
    return nc


def _prep_inputs(x, qkv_w, q_bias, v_bias, rpb_table, proj_w, proj_b, rel_pos_index):
    bf16 = ml_dtypes.bfloat16
    x = np.asarray(x, np.float32)
    qkv_w = np.asarray(qkv_w, np.float32)
    q_bias = np.asarray(q_bias, np.float32)
    v_bias = np.asarray(v_bias, np.float32)
    rpb_table = np.asarray(rpb_table, np.float32)
    proj_w = np.asarray(proj_w, np.float32)
    proj_b = np.asarray(proj_b, np.float32)
    rel_pos_index = np.asarray(rel_pos_index)

    qkv_wt = qkv_w.T.copy()  # [768, 2304]
    qkv_wt[:, :DIM] *= SCALE
    qkv_wt = np.ascontiguousarray(qkv_wt, dtype=bf16)

    qb = (q_bias * SCALE).reshape(12, 64, 1).astype(np.float32)

    proj_wt = np.ascontiguousarray(proj_w.T, dtype=bf16)
    pb_eff = np.tile((proj_b + proj_w @ v_bias).reshape(1, DIM), (128, 1)).astype(bf16)

    # bias[h, n, m] = rpb_table[rel_pos_index[n, m], h]; store exp() as
    # [m-chunk, m-in-chunk, h*197 + n]
    bias_nmh = rpb_table[rel_pos_index]  # [n, m, h]
    er = np.exp(bias_nmh.transpose(1, 2, 0))  # [m, h, n]
    er = er.reshape(NTOK, HEADS * NTOK)
    er_pad = np.ones((256, HEADS * NTOK), np.float32)
    er_pad[:NTOK] = er
    exp_rpb = np.ascontiguousarray(er_pad.reshape(2, 128, HEADS * NTOK), dtype=bf16)

    shared = {
        "qkv_wt": qkv_wt,
        "qb": qb,
        "proj_wt": proj_wt,
        "pb": pb_eff,
        "exp_rpb": exp_rpb,
    }
    in_maps = []
    for c in range(NCORES):
        xc = x[c * BS : (c + 1) * BS].reshape(NT, DIM)
        xp = np.zeros((NTP, DIM), bf16)
        xp[:NT] = xc.astype(bf16)
        in_maps.append({"x": xp, **shared})
    return in_maps


def run(inputs, trace=False):
    """Build (cached), run on 8 cores, return (y_full, BassKernelResults)."""
    from concourse.bass_utils import run_bass_kernel_spmd

    if "nc" not in _CACHE:
        _CACHE["nc"] = _build_bass()
    nc = _CACHE["nc"]
    in_maps = _prep_inputs(**{k: inputs[k] for k in (
        "x", "qkv_w", "q_bias", "v_bias", "rpb_table", "proj_w", "proj_b",
        "rel_pos_index")})
    try:
        res = run_bass_kernel_spmd(
            nc, in_maps, core_ids=list(range(NCORES)), trace=trace
        )
    except ModuleNotFoundError:
        # NTFF profile hook unavailable in this container; run untraced
        res = run_bass_kernel_spmd(
            nc, in_maps, core_ids=list(range(NCORES)), trace=False
        )
    y = np.concatenate(
        [res.results[c]["y"].reshape(BS, NTOK, DIM) for c in range(NCORES)], axis=0
    )
    return np.ascontiguousarray(y, np.float32), res


def kernel(**inputs) -> np.ndarray:
    y, _ = run(inputs, trace=False)
    return y



# revision 22
# speedup vs baseline: 1.2188x; 1.2188x over previous
"""ViT attention block (B=64, N=197, H=12, hd=64, D=768) on 8 trn2 NeuronCores.

Pure data-parallel: 8 batches per core.  Per-core pipeline (all matmuls bf16,
fp32 PSUM accumulation):

  xT  <- DMA-transpose(x bf16)                        [768, 1600]
  q,k <- W_qk @ xT   M=128 tiles (two heads per tile) -> qkT[64, 24 heads, tok]
         eviction splits the PSUM halves (partition-shifted DVE/ACT ops);
         q pre-scaled by 1/8 on host, +q_bias folded into eviction
  v   <- xT.T @ W_v  (natural layout, batch-aligned M-tiles); evicted into
         vsb[*, slot, head, 0:64] with columns 64:128 memset to 1.0 so each
         head's lhsT is [v_h | ones] (M=128)
  per batch b, head-pair g, key-chunk mc:
    S[m,n]   = I @ rpb + k^T q      (identity matmul preloads the relative
                                     position bias into the PSUM accumulation
                                     group; no separate elementwise add)
    e2       = exp(S)               (ACT, straight into e2 SBUF bf16)
    O[0:128] = [v_h | ones]^T @ e2  (rows 0:64 = attnout, 64:128 = softmax
                                     sums replicated 64x)
    B        = 1/O[64:128]          (DVE reciprocal_approx_fast, one op per
                                     head pair)
    outT     = O[0:64] * B          (DVE, head pairs stacked into partitions
                                     0:64 / 64:128 of outT[128, 6, tok])
  y = outT.T @ proj_wT (6 K=128 chunks) + proj_b row added during eviction
      (v_bias pre-folded into proj_b on host: softmax rows sum to 1)

Hardware constraints honored: every PE operand (lhsT/rhs) and matmul PSUM
output sits at base_partition 0 (upper-quadrant streaming crashes); one
accumulation group per PSUM bank; DMA-transposes barrier-separated from DMA
copies (xbar hang); output stores via gpsimd SWDGE; gpsimd cannot access
PSUM.  DVE/ACT partition-base shifts (PSUM[64:128] -> SBUF[0:64] and
SBUF[64:128] writes) verified on HW.
"""

import os
import sys

import numpy as np

for _p in ("/opt/trn_rl_repo", os.path.expanduser("~/.axon_site/_ro/trn_rl_repo")):
    if os.path.isdir(_p) and _p not in sys.path:
        sys.path.insert(0, _p)

import ml_dtypes  # noqa: E402

B = 64
NTOK = 197
DIM = 768
HEADS = 12
HD = 64
NCORES = 8
BS = B // NCORES  # 8 batches per core
NT = BS * NTOK  # 1576 real tokens per core
NTP = 1600  # padded tokens (mult of 16 for xbar; 12x128 + 64)
SCALE = HD ** -0.5

_CACHE = {}


def _build_bass(stop_after=None):
    stop_after = stop_after or os.environ.get("K_STOP_AFTER", "")
    import concourse.mybir as mybir
    import concourse.tile as tile
    from concourse import bacc

    f32 = mybir.dt.float32
    bf16 = mybir.dt.bfloat16
    EXP = mybir.ActivationFunctionType.Exp

    nc = bacc.Bacc(
        "TRN2", target_bir_lowering=False, debug=False,
        num_devices=int(os.environ.get("K_NDEV", str(NCORES))),
    )

    x_d = nc.dram_tensor("x", [NTP, DIM], bf16, kind="ExternalInput")
    qkvw_d = nc.dram_tensor("qkv_wt", [DIM, 3 * DIM], bf16, kind="ExternalInput")
    qb_d = nc.dram_tensor("qb", [12, 64, 1], f32, kind="ExternalInput")
    projw_d = nc.dram_tensor("proj_wt", [DIM, DIM], bf16, kind="ExternalInput")
    pb_d = nc.dram_tensor("pb", [128, DIM], bf16, kind="ExternalInput")
    rpb_d = nc.dram_tensor("rpb", [2, 128, HEADS * NTOK], bf16, kind="ExternalInput")
    id_d = nc.dram_tensor("ident", [128, 128], bf16, kind="ExternalInput")
    y_d = nc.dram_tensor("y", [NT, DIM], f32, kind="ExternalOutput")

    NTILES = [(0, 512), (512, 512), (1024, 512), (1536, 64)]
    VTILES = [(0, 512), (512, 256)]

    with tile.TileContext(nc, linearize=bool(os.environ.get("K_LINEARIZE"))) as tc:
        with (
            tc.tile_pool(name="consts", bufs=1) as consts,
            tc.tile_pool(name="acts", bufs=1) as acts,
        ):
            # ---- constants ----
            projw = consts.tile([128, 6, DIM], bf16)  # head-pair K chunks
            rpb = consts.tile([128, 2, HEADS * NTOK], bf16)
            qb = consts.tile([64, 12, 1], f32)
            pb = consts.tile([128, DIM], bf16)
            ident = consts.tile([128, 128], bf16)

            # persistent activations
            qkT = acts.tile([64, 2 * HEADS, NTP], bf16)  # q heads 0-11, k 12-23
            vsb = acts.tile([128, 2 * BS, HEADS, 128], bf16)  # [ones | v_h]

            projw_v = projw_d[:].rearrange("(k p) n -> p k n", p=128)
            for k in range(6):
                nc.sync.dma_start(out=projw[:, k, :], in_=projw_v[:, k, :])
            for mc in range(2):
                nc.sync.dma_start(out=rpb[:, mc, :], in_=rpb_d[mc, :, :])
            nc.sync.dma_start(out=qb[:, :, :], in_=qb_d[:].rearrange("k p o -> p k o"))
            nc.sync.dma_start(out=pb[:, :], in_=pb_d[:, :])
            nc.sync.dma_start(out=ident[:, :], in_=id_d[:, :])
            nc.vector.memset(vsb[:, :, :, 0:64], 1.0)

            do_qkv = stop_after != "load"
            do_attn = do_qkv and stop_after != "qkv"
            do_proj = do_attn and stop_after != "attn"

            # ---- qkv projections ----
            with (
                tc.tile_pool(name="ldp", bufs=1) as ldp,
                tc.tile_pool(name="ps_qk", bufs=1, space="PSUM") as ps_qk,
                tc.tile_pool(name="ps_v", bufs=2, space="PSUM") as ps_v,
            ):
                qkvw = ldp.tile([128, 6, 3 * DIM], bf16)
                xt = ldp.tile([128, 6, NTP], bf16)  # x transposed [c, tok]
                for k in range(6):
                    nc.sync.dma_start_transpose(
                        out=xt[:, k, :], in_=x_d[:, k * 128 : (k + 1) * 128]
                    )
                tc.strict_bb_all_engine_barrier()
                qkvw_v = qkvw_d[:].rearrange("(k p) n -> p k n", p=128)
                for k in range(6):
                    nc.sync.dma_start(out=qkvw[:, k, :], in_=qkvw_v[:, k, :])

                # q,k: 12 M=128 tiles (two heads each); k-outer loop amortizes
                # LDWEIGHTS over the 4 n-tiles
                for t in range(12 if do_qkv else 0):
                    pss = [
                        ps_qk.tile([128, 512], f32, name=f"pss{j}")
                        for j in range(4)
                    ]
                    for k in range(6):
                        for j, (noff, nsz) in enumerate(NTILES):
                            nc.tensor.matmul(
                                pss[j][:, :nsz],
                                qkvw[:, k, t * 128 : (t + 1) * 128],
                                xt[:, k, noff : noff + nsz],
                                start=(k == 0),
                                stop=(k == 5),
                            )
                    for j, (noff, nsz) in enumerate(NTILES):
                        if t < 6:  # q: add bias (pre-scaled on host)
                            nc.vector.tensor_scalar_add(
                                qkT[:, 2 * t, noff : noff + nsz],
                                pss[j][0:64, :nsz],
                                qb[:, 2 * t, 0:1],
                            )
                            nc.vector.tensor_scalar_add(
                                qkT[:, 2 * t + 1, noff : noff + nsz],
                                pss[j][64:128, :nsz],
                                qb[:, 2 * t + 1, 0:1],
                            )
                        else:  # k: plain copies
                            h0 = 2 * (t - 6)
                            nc.scalar.copy(
                                qkT[:, HEADS + h0, noff : noff + nsz],
                                pss[j][0:64, :nsz],
                            )
                            nc.scalar.copy(
                                qkT[:, HEADS + h0 + 1, noff : noff + nsz],
                                pss[j][64:128, :nsz],
                            )

                for b in range(BS if do_qkv else 0):
                    for mc in range(2):
                        msz = 128 if mc == 0 else NTOK - 128
                        toff = b * NTOK + mc * 128
                        psv = ps_v.tile([128, DIM], f32)
                        for k in range(6):
                            for noff, nsz in VTILES:
                                nc.tensor.matmul(
                                    psv[:msz, noff : noff + nsz],
                                    xt[:, k, toff : toff + msz],
                                    qkvw[:, k, 1536 + noff : 1536 + noff + nsz],
                                    start=(k == 0),
                                    stop=(k == 5),
                                )
                        nc.scalar.copy(
                            vsb[:msz, b * 2 + mc, :, 64:128],
                            psv[:msz, :].rearrange("p (h d) -> p h d", d=64),
                        )

            otp_cm = tc.tile_pool(name="otp", bufs=1)
            otp = otp_cm.__enter__()
            outT = otp.tile([128, 6, NTP], bf16)  # attn out, head pairs stacked

            if stop_after == "qkv":
                nc.gpsimd.dma_start(out=y_d[0:64, :], in_=qkT[:, 0, 0:DIM])
                nc.gpsimd.dma_start(out=y_d[64:128, :], in_=qkT[:, 1, 0:DIM])
                nc.gpsimd.dma_start(
                    out=y_d[128:256, :].rearrange("p (h d) -> p h d", d=64),
                    in_=vsb[:, 0, :, 64:128],
                )
                nc.gpsimd.dma_start(out=y_d[256:320, :], in_=qkT[:, 12, 0:DIM])
                nc.gpsimd.dma_start(out=y_d[320:384, :], in_=qkT[:, 13, 0:DIM])
                nc.gpsimd.dma_start(
                    out=y_d[384:453, :].rearrange("p (h d) -> p h d", d=64),
                    in_=vsb[0:69, 1, :, 64:128],
                )

            # ---- attention ----
            NB = int(os.environ.get("K_NB", str(BS)))
            probes = {}
            if stop_after == "attn":
                probes["S"] = otp.tile([128, 2, NTOK], f32, name="probe_S")
                probes["e2"] = otp.tile([128, 2, NTOK], f32, name="probe_e2")
                probes["O"] = otp.tile([128, 2, NTOK], f32, name="probe_O")
                probes["Bt"] = otp.tile([64, 2, NTOK], f32, name="probe_Bt")
            if do_attn:
             with (
                tc.tile_pool(name="e2p", bufs=2) as e2p,
                tc.tile_pool(name="bp", bufs=2) as bp,
                tc.tile_pool(name="ps_s", bufs=2, space="PSUM") as ps_s,
                tc.tile_pool(name="ps_o", bufs=2, space="PSUM") as ps_o,
            ):
                def emit_o(b, g, e2, Bt):
                    """O matmuls + recip + normalize for head pair g of batch b."""
                    tb = b * NTOK
                    O = ps_o.tile([128, 2, 512], f32)
                    for hh in range(2):
                        h = 2 * g + hh
                        for mc in range(2):
                            msz = 128 if mc == 0 else NTOK - 128
                            nc.tensor.matmul(
                                O[:, hh, 0:NTOK],
                                vsb[:msz, b * 2 + mc, h, :],
                                e2[:msz, mc, h * NTOK : (h + 1) * NTOK],
                                start=(mc == 0),
                                stop=(mc == 1),
                            )
                    if probes and b == 0 and g == 0:
                        nc.vector.tensor_copy(probes["O"][:, :, :], O[:, :, 0:NTOK])
                    nc.vector.reciprocal_approx_fast(
                        out=Bt[0:64, :].rearrange("p (s n) -> p s n", n=NTOK)[
                            :, 2 * g : 2 * g + 2, :
                        ],
                        in_=O[0:64, :, 0:NTOK],
                    )
                    if probes and b == 0 and g == 0:
                        nc.vector.tensor_copy(
                            probes["Bt"][:, :, :],
                            Bt[0:64, 0 : 2 * NTOK].rearrange("p (s n) -> p s n", s=2),
                        )
                    for hh in range(2):
                        nc.vector.tensor_mul(
                            outT[hh * 64 : (hh + 1) * 64, g, tb : tb + NTOK],
                            O[64:128, hh, 0:NTOK],
                            Bt[0:64, (2 * g + hh) * NTOK : (2 * g + hh + 1) * NTOK],
                        )

                prev = None
                for b in range(NB):
                    tb = b * NTOK
                    e2 = e2p.tile([128, 2, HEADS * NTOK], bf16)
                    Bt = bp.tile([64, HEADS * NTOK], f32)
                    for g in range(6):
                        for mc in range(2):
                            msz = 128 if mc == 0 else NTOK - 128
                            S = ps_s.tile([128, 1024], f32)
                            for hh in range(2):
                                h = 2 * g + hh
                                nc.tensor.matmul(
                                    S[:msz, hh * 512 : hh * 512 + NTOK],
                                    ident[:msz, :msz],
                                    rpb[:msz, mc, h * NTOK : (h + 1) * NTOK],
                                    start=True,
                                    stop=False,
                                )
                                nc.tensor.matmul(
                                    S[:msz, hh * 512 : hh * 512 + NTOK],
                                    qkT[:, HEADS + h, tb + mc * 128 : tb + mc * 128 + msz],
                                    qkT[:, h, tb : tb + NTOK],
                                    start=False,
                                    stop=True,
                                )
                            nc.scalar.activation(
                                e2[:msz, mc, 2 * g * NTOK : (2 * g + 2) * NTOK]
                                .rearrange("p (s n) -> p s n", s=2),
                                S[:msz, :].rearrange("p (s n) -> p s n", s=2)[
                                    :, :, :NTOK
                                ],
                                EXP,
                            )
                            if probes and b == 0 and g == 0 and mc == 0:
                                nc.vector.tensor_copy(
                                    probes["S"][:, :, :],
                                    S[:, :].rearrange("p (s n) -> p s n", s=2)[
                                        :, :, :NTOK
                                    ],
                                )
                                nc.scalar.copy(
                                    probes["e2"][:, :, :],
                                    e2[:, 0, 0 : 2 * NTOK].rearrange(
                                        "p (s n) -> p s n", s=2
                                    ),
                                )
                        if prev is not None:
                            emit_o(*prev)
                        prev = (b, g, e2, Bt)
                if prev is not None:
                    emit_o(*prev)

            if stop_after == "attn":
                nc.gpsimd.dma_start(out=y_d[0:128, :], in_=outT[:, 0, 0:DIM])
                nc.gpsimd.dma_start(
                    out=y_d[128:256, 0 : 2 * NTOK].rearrange(
                        "p (s n) -> p s n", s=2
                    ),
                    in_=probes["S"][:, :, :],
                )
                nc.gpsimd.dma_start(
                    out=y_d[256:384, 0 : 2 * NTOK].rearrange(
                        "p (s n) -> p s n", s=2
                    ),
                    in_=probes["e2"][:, :, :],
                )
                nc.gpsimd.dma_start(
                    out=y_d[384:512, 0 : 2 * NTOK].rearrange(
                        "p (s n) -> p s n", s=2
                    ),
                    in_=probes["O"][:, :, :],
                )
                nc.gpsimd.dma_start(
                    out=y_d[512:576, 0 : 2 * NTOK].rearrange(
                        "p (s n) -> p s n", s=2
                    ),
                    in_=probes["Bt"][:, :, :],
                )

            # ---- output projection ----
            if do_proj:
             with (
                tc.tile_pool(name="yp", bufs=2) as yp,
                tc.tile_pool(name="ps_y", bufs=2, space="PSUM") as ps_y,
             ):
                for m in range(13):
                    moff = m * 128
                    msz = min(128, NTP - moff)
                    real = min(128, NT - moff)
                    Y = ps_y.tile([128, DIM], f32)
                    for kp in range(6):
                        for noff, nsz in VTILES:
                            nc.tensor.matmul(
                                Y[:msz, noff : noff + nsz],
                                outT[:, kp, moff : moff + msz],
                                projw[:, kp, noff : noff + nsz],
                                start=(kp == 0),
                                stop=(kp == 5),
                            )
                    ysb = yp.tile([128, DIM], f32)
                    nc.vector.tensor_add(ysb[:msz, :], Y[:msz, :], pb[:msz, :])
                    nc.gpsimd.dma_start(out=y_d[moff : moff + real, :], in_=ysb[:real, :])
            otp_cm.__exit__(None, None, None)

    nc.compile()
    return nc


def _prep_inputs(x, qkv_w, q_bias, v_bias, rpb_table, proj_w, proj_b, rel_pos_index):
    bf16 = ml_dtypes.bfloat16
    x = np.asarray(x, np.float32)
    qkv_w = np.asarray(qkv_w, np.float32)
    q_bias = np.asarray(q_bias, np.float32)
    v_bias = np.asarray(v_bias, np.float32)
    rpb_table = np.asarray(rpb_table, np.float32)
    proj_w = np.asarray(proj_w, np.float32)
    proj_b = np.asarray(proj_b, np.float32)
    rel_pos_index = np.asarray(rel_pos_index)

    qkv_wt = qkv_w.T.copy()  # [768, 2304]
    qkv_wt[:, :DIM] *= SCALE
    qkv_wt = np.ascontiguousarray(qkv_wt, dtype=bf16)

    qb = (q_bias * SCALE).reshape(12, 64, 1).astype(np.float32)

    proj_wt = np.ascontiguousarray(proj_w.T, dtype=bf16)
    pb_eff = np.tile((proj_b + proj_w @ v_bias).reshape(1, DIM), (128, 1)).astype(bf16)

    # bias[h, n, m] = rpb_table[rel_pos_index[n, m], h]; store raw (additive)
    # as [m-chunk, m-in-chunk, h*197 + n]
    bias_nmh = rpb_table[rel_pos_index]  # [n, m, h]
    er = bias_nmh.transpose(1, 2, 0)  # [m, h, n]
    er = er.reshape(NTOK, HEADS * NTOK)
    er_pad = np.zeros((256, HEADS * NTOK), np.float32)
    er_pad[:NTOK] = er
    rpb = np.ascontiguousarray(er_pad.reshape(2, 128, HEADS * NTOK), dtype=bf16)

    ident = np.eye(128, dtype=bf16)

    shared = {
        "qkv_wt": qkv_wt,
        "qb": qb,
        "proj_wt": proj_wt,
        "pb": pb_eff,
        "rpb": rpb,
        "ident": ident,
    }
    in_maps = []
    for c in range(NCORES):
        xc = x[c * BS : (c + 1) * BS].reshape(NT, DIM)
        xp = np.zeros((NTP, DIM), bf16)
        xp[:NT] = xc.astype(bf16)
        in_maps.append({"x": xp, **shared})
    return in_maps


def run(inputs, trace=False):
    """Build (cached), run on 8 cores, return (y_full, BassKernelResults)."""
    from concourse.bass_utils import run_bass_kernel_spmd

    if "nc" not in _CACHE:
        _CACHE["nc"] = _build_bass()
    nc = _CACHE["nc"]
    in_maps = _prep_inputs(**{k: inputs[k] for k in (
        "x", "qkv_w", "q_bias", "v_bias", "rpb_table", "proj_w", "proj_b",
        "rel_pos_index")})
    try:
        res = run_bass_kernel_spmd(
            nc, in_maps, core_ids=list(range(NCORES)), trace=trace
        )
    except ModuleNotFoundError:
        # NTFF profile hook unavailable in this container; run untraced
        res = run_bass_kernel_spmd(
            nc, in_maps, core_ids=list(range(NCORES)), trace=False
        )
    y = np.concatenate(
        [res.results[c]["y"].reshape(BS, NTOK, DIM) for c in range(NCORES)], axis=0
    )
    return np.ascontiguousarray(y, np.float32), res


def kernel(**inputs) -> np.ndarray:
    y, _ = run(inputs, trace=False)
    return y


# revision 33
# speedup vs baseline: 1.4030x; 1.1511x over previous
"""ViT attention block (B=64, N=197, H=12, hd=64, D=768) on 8 trn2 NeuronCores.

Pure data-parallel: 8 batches per core.  Single interleaved PE stream to keep
the HAM clock-gate warm (idle/low-utilization PE re-throttles to 1.2 GHz):

  prelude: xT <- DMA-transpose(x); q,k <- W_qk @ xT as 12 M=128 tiles
           (two heads per tile, k-outer loop, 4 live PSUM banks); v(b0,b1)
  windows: attention for batch b interleaved with "big" matmul units that
           keep the PE array streaming densely: v(b2..b7) during b0..b2,
           output-projection m-tiles during b3..b7 (+3 in the tail).

Attention per (batch b, head pair g, key chunk mc):
  S    = I @ rpb + k^T q      identity matmul preloads the relative-position
                              bias into the PSUM accumulation group
  e2   = exp(S)               ACT, straight to bf16 SBUF (per-(b,g) tile)
  O    = [ones | v_h]^T @ e2  M=128: rows 0:64 = softmax sums (replicated),
                              rows 64:128 = unnormalized attention out
  B    = 1/O[0:64]            DVE reciprocal_approx_fast (input must be at
                              partition base 0 - custom DVE ops ignore the
                              AP base_partition)
  outT = O[64:128] * B        DVE, head pairs stacked into partitions 0:64 /
                              64:128 of outT[128, 6, tok]
Projection: y = outT.T @ proj_wT (6 K=128 chunks) accumulated on top of a
K=1 ones-row matmul that preloads proj_b (v_bias folded in on host); evicted
by ACT copy, stored by gpsimd SWDGE.

Hardware constraints honored: PE operands and matmul PSUM outputs at
base_partition 0; one accumulation group per PSUM bank; DMA-transposes
barrier-separated from DMA copies; gpsimd cannot access PSUM; q pre-scaled
by 1/8 on host.  DVE/ACT partition-base shifts verified on HW.
"""

import os
import sys

import numpy as np

for _p in ("/opt/trn_rl_repo", os.path.expanduser("~/.axon_site/_ro/trn_rl_repo")):
    if os.path.isdir(_p) and _p not in sys.path:
        sys.path.insert(0, _p)

import ml_dtypes  # noqa: E402

B = 64
NTOK = 197
DIM = 768
HEADS = 12
HD = 64
NCORES = 8
BS = B // NCORES  # 8 batches per core
NT = BS * NTOK  # 1576 real tokens per core
NTP = 1600  # padded tokens (mult of 16 for xbar; 12x128 + 64)
SCALE = HD ** -0.5

_CACHE = {}


def _build_bass(stop_after=None):
    stop_after = stop_after or os.environ.get("K_STOP_AFTER", "")
    import concourse.mybir as mybir
    import concourse.tile as tile
    from concourse import bacc

    f32 = mybir.dt.float32
    bf16 = mybir.dt.bfloat16
    EXP = mybir.ActivationFunctionType.Exp

    nc = bacc.Bacc(
        "TRN2", target_bir_lowering=False, debug=False,
        num_devices=int(os.environ.get("K_NDEV", str(NCORES))),
    )

    x_d = nc.dram_tensor("x", [NTP, DIM], bf16, kind="ExternalInput")
    qkvw_d = nc.dram_tensor("qkv_wt", [DIM, 3 * DIM], bf16, kind="ExternalInput")
    qb_d = nc.dram_tensor("qb", [12, 64, 1], f32, kind="ExternalInput")
    projw_d = nc.dram_tensor("proj_wt", [DIM, DIM], bf16, kind="ExternalInput")
    pb_d = nc.dram_tensor("pb", [1, DIM], bf16, kind="ExternalInput")
    rpb_d = nc.dram_tensor("rpb", [2, 128, HEADS * NTOK], bf16, kind="ExternalInput")
    id_d = nc.dram_tensor("ident", [128, 128], bf16, kind="ExternalInput")
    y_d = nc.dram_tensor("y", [NT, DIM], f32, kind="ExternalOutput")

    NTILES = [(0, 512), (512, 512), (1024, 512), (1536, 64)]
    VTILES = [(0, 512), (512, 256)]

    with tile.TileContext(nc, linearize=bool(os.environ.get("K_LINEARIZE"))) as tc:
        with (
            tc.tile_pool(name="consts", bufs=1) as consts,
            tc.tile_pool(name="acts", bufs=1) as acts,
        ):
            # ---- constants ----
            projw = consts.tile([128, 6, DIM], bf16)  # head-pair K chunks
            rpb = consts.tile([128, 2, HEADS * NTOK], bf16)
            qb = consts.tile([64, 12, 1], f32)
            pb = consts.tile([1, DIM], bf16)
            ident = consts.tile([128, 128], bf16)
            ones1 = consts.tile([1, 128], bf16)

            # persistent activations
            qkT = acts.tile([64, 2 * HEADS, NTP], bf16)  # q heads 0-11, k 12-23
            vsb = acts.tile([128, 2 * BS, HEADS, 128], bf16)  # [ones | v_h]

            xp_cm = tc.tile_pool(name="xp", bufs=1)
            xp = xp_cm.__enter__()
            xt = xp.tile([128, 6, NTP], bf16)  # x transposed [c, tok]
            vw = xp.tile([128, 6, DIM], bf16)
            wqk_cm = tc.tile_pool(name="wqk", bufs=1)
            wqk = wqk_cm.__enter__()
            qkvw = wqk.tile([128, 6, 2 * DIM], bf16)

            for k in range(6):
                nc.sync.dma_start_transpose(
                    out=xt[:, k, :], in_=x_d[:, k * 128 : (k + 1) * 128]
                )
            tc.strict_bb_all_engine_barrier()
            qkvw_v = qkvw_d[:].rearrange("(k p) n -> p k n", p=128)
            for k in range(6):
                nc.sync.dma_start(out=qkvw[:, k, :], in_=qkvw_v[:, k, 0 : 2 * DIM])
            nc.sync.dma_start(out=qb[:, :, :], in_=qb_d[:].rearrange("k p o -> p k o"))
            for k in range(6):
                nc.sync.dma_start(out=vw[:, k, :], in_=qkvw_v[:, k, 2 * DIM : 3 * DIM])
            for mc in range(2):
                nc.sync.dma_start(out=rpb[:, mc, :], in_=rpb_d[mc, :, :])
            nc.sync.dma_start(out=pb[:, :], in_=pb_d[:, :])
            nc.sync.dma_start(out=ident[:, :], in_=id_d[:, :])
            projw_v = projw_d[:].rearrange("(k p) n -> p k n", p=128)
            for k in range(6):
                nc.sync.dma_start(out=projw[:, k, :], in_=projw_v[:, k, :])
            nc.vector.memset(vsb[:, :, :, 0:64], 1.0)
            nc.vector.memset(ones1[:, :], 1.0)

            do_qkv = stop_after != "load"
            do_attn = do_qkv and stop_after != "qkv"
            do_proj = do_attn and stop_after != "attn"

            ps_v_cm = tc.tile_pool(name="ps_v", bufs=1, space="PSUM")
            ps_v = ps_v_cm.__enter__()
            ps_qk_cm = tc.tile_pool(name="ps_qk", bufs=1, space="PSUM")
            ps_qk = ps_qk_cm.__enter__()

            def emit_qk(t):
                pss = [
                    ps_qk.tile([128, 512], f32, name=f"pss{j}") for j in range(4)
                ]
                for k in range(6):
                    for j, (noff, nsz) in enumerate(NTILES):
                        nc.tensor.matmul(
                            pss[j][:, :nsz],
                            qkvw[:, k, t * 128 : (t + 1) * 128],
                            xt[:, k, noff : noff + nsz],
                            start=(k == 0),
                            stop=(k == 5),
                        )
                for j, (noff, nsz) in enumerate(NTILES):
                    if t < 6:  # q: add bias (pre-scaled on host)
                        nc.vector.tensor_scalar_add(
                            qkT[:, 2 * t, noff : noff + nsz],
                            pss[j][0:64, :nsz],
                            qb[:, 2 * t, 0:1],
                        )
                        nc.vector.tensor_scalar_add(
                            qkT[:, 2 * t + 1, noff : noff + nsz],
                            pss[j][64:128, :nsz],
                            qb[:, 2 * t + 1, 0:1],
                        )
                    else:  # k: plain copies
                        h0 = 2 * (t - 6)
                        nc.scalar.copy(
                            qkT[:, HEADS + h0, noff : noff + nsz],
                            pss[j][0:64, :nsz],
                        )
                        nc.scalar.copy(
                            qkT[:, HEADS + h0 + 1, noff : noff + nsz],
                            pss[j][64:128, :nsz],
                        )

            def emit_v(b, mc):
                msz = 128 if mc == 0 else NTOK - 128
                toff = b * NTOK + mc * 128
                psv = ps_v.tile([128, DIM], f32, name="psv")
                for k in range(6):
                    for noff, nsz in VTILES:
                        nc.tensor.matmul(
                            psv[:msz, noff : noff + nsz],
                            xt[:, k, toff : toff + msz],
                            vw[:, k, noff : noff + nsz],
                            start=(k == 0),
                            stop=(k == 5),
                        )
                nc.scalar.copy(
                    vsb[:msz, b * 2 + mc, :, 64:128],
                    psv[:msz, :].rearrange("p (h d) -> p h d", d=64),
                )

            # ---- prelude: all of q,k + v(b0,b1) ----
            for t in range(12 if do_qkv else 0):
                emit_qk(t)
            for b in range(2 if do_qkv else 0):
                for mc in range(2):
                    emit_v(b, mc)
            ps_qk_cm.__exit__(None, None, None)
            wqk_cm.__exit__(None, None, None)

            otp_cm = tc.tile_pool(name="otp", bufs=1)
            otp = otp_cm.__enter__()
            outT = otp.tile([128, 6, NTP], bf16)  # attn out, head pairs stacked

            if stop_after == "qkv":
                nc.gpsimd.dma_start(out=y_d[0:64, :], in_=qkT[:, 0, 0:DIM])
                nc.gpsimd.dma_start(out=y_d[64:128, :], in_=qkT[:, 1, 0:DIM])
                nc.gpsimd.dma_start(
                    out=y_d[128:256, :].rearrange("p (h d) -> p h d", d=64),
                    in_=vsb[:, 0, :, 64:128],
                )
                nc.gpsimd.dma_start(out=y_d[256:320, :], in_=qkT[:, 12, 0:DIM])
                nc.gpsimd.dma_start(out=y_d[320:384, :], in_=qkT[:, 13, 0:DIM])
                nc.gpsimd.dma_start(
                    out=y_d[384:453, :].rearrange("p (h d) -> p h d", d=64),
                    in_=vsb[0:69, 1, :, 64:128],
                )

            # ---- attention interleaved with v(b2..b7) and projection ----
            probes = {}
            if stop_after == "attn":
                probes["S"] = otp.tile([128, 2, NTOK], f32, name="probe_S")
                probes["e2"] = otp.tile([128, 2, NTOK], f32, name="probe_e2")
                probes["O"] = otp.tile([128, 2, NTOK], f32, name="probe_O")
                probes["Bt"] = otp.tile([64, 2, NTOK], f32, name="probe_Bt")

            e2p_cm = tc.tile_pool(name="e2p", bufs=3)
            e2p = e2p_cm.__enter__()
            bp_cm = tc.tile_pool(name="bp", bufs=2)
            bp = bp_cm.__enter__()
            ps_s_cm = tc.tile_pool(name="ps_s", bufs=1, space="PSUM")
            ps_s = ps_s_cm.__enter__()
            ps_o_cm = tc.tile_pool(name="ps_o", bufs=1, space="PSUM")
            ps_o = ps_o_cm.__enter__()
            yp_cm = tc.tile_pool(name="yp", bufs=2)
            yp = yp_cm.__enter__()
            ps_y_cm = tc.tile_pool(name="ps_y", bufs=1, space="PSUM")
            ps_y = ps_y_cm.__enter__()

            def emit_s(b, g, mc, e2):
                tb = b * NTOK
                msz = 128 if mc == 0 else NTOK - 128
                S = ps_s.tile([128, 1024], f32, name="S")
                for hh in range(2):
                    h = 2 * g + hh
                    nc.tensor.matmul(
                        S[:msz, hh * 512 : hh * 512 + NTOK],
                        ident[:msz, :msz],
                        rpb[:msz, mc, h * NTOK : (h + 1) * NTOK],
                        start=True,
                        stop=False,
                    )
                    nc.tensor.matmul(
                        S[:msz, hh * 512 : hh * 512 + NTOK],
                        qkT[:, HEADS + h, tb + mc * 128 : tb + mc * 128 + msz],
                        qkT[:, h, tb : tb + NTOK],
                        start=False,
                        stop=True,
                    )
                nc.scalar.activation(
                    e2[:msz, mc, :, :],
                    S[:msz, :].rearrange("p (s n) -> p s n", s=2)[:, :, :NTOK],
                    EXP,
                )
                if probes and b == 0 and g == 0 and mc == 0:
                    nc.vector.tensor_copy(
                        probes["S"][:, :, :],
                        S[:, :].rearrange("p (s n) -> p s n", s=2)[:, :, :NTOK],
                    )
                    nc.scalar.copy(probes["e2"][:, :, :], e2[:, 0, :, :])

            def emit_o(b, g, e2, Bt):
                tb = b * NTOK
                O = ps_o.tile([128, 2, 512], f32, name="O")
                for hh in range(2):
                    h = 2 * g + hh
                    for mc in range(2):
                        msz = 128 if mc == 0 else NTOK - 128
                        nc.tensor.matmul(
                            O[:, hh, 0:NTOK],
                            vsb[:msz, b * 2 + mc, h, :],
                            e2[:msz, mc, hh, :],
                            start=(mc == 0),
                            stop=(mc == 1),
                        )
                if probes and b == 0 and g == 0:
                    nc.vector.tensor_copy(probes["O"][:, :, :], O[:, :, 0:NTOK])
                nc.vector.reciprocal_approx_fast(
                    out=Bt[:, :, :], in_=O[0:64, :, 0:NTOK]
                )
                if probes and b == 0 and g == 0:
                    nc.vector.tensor_copy(probes["Bt"][:, :, :], Bt[:, :, :])
                for hh in range(2):
                    nc.vector.tensor_mul(
                        outT[hh * 64 : (hh + 1) * 64, g, tb : tb + NTOK],
                        O[64:128, hh, 0:NTOK],
                        Bt[:, hh, :],
                    )

            def emit_proj(m):
                moff = m * 128
                msz = min(128, NTP - moff)
                real = min(128, NT - moff)
                Y = ps_y.tile([128, DIM], f32, name="Y")
                for noff, nsz in VTILES:
                    nc.tensor.matmul(
                        Y[:msz, noff : noff + nsz],
                        ones1[0:1, 0:msz],
                        pb[0:1, noff : noff + nsz],
                        start=True,
                        stop=False,
                    )
                for kp in range(6):
                    for noff, nsz in VTILES:
                        nc.tensor.matmul(
                            Y[:msz, noff : noff + nsz],
                            outT[:, kp, moff : moff + msz],
                            projw[:, kp, noff : noff + nsz],
                            start=False,
                            stop=(kp == 5),
                        )
                ysb = yp.tile([128, DIM], f32, name="ysb")
                nc.scalar.copy(ysb[:msz, :], Y[:msz, :])
                nc.gpsimd.dma_start(
                    out=y_d[moff : moff + real, :], in_=ysb[:real, :]
                )

            # big-unit schedule per batch window (emission order):
            # v units must precede their batch's O units; proj m-tile needs
            # all batches overlapping tokens [128m, 128m+128) fully emitted.
            big_sched = {
                0: [("v", 2, 0), ("v", 2, 1), ("v", 3, 0), ("v", 3, 1)],
                1: [("v", 4, 0), ("v", 4, 1), ("v", 5, 0), ("v", 5, 1)],
                2: [("v", 6, 0), ("v", 6, 1), ("v", 7, 0), ("v", 7, 1)],
                3: [("p", 0), ("p", 1), ("p", 2), ("p", 3)],
                4: [("p", 4), ("p", 5)],
                5: [("p", 6)],
                6: [("p", 7), ("p", 8)],
                7: [("p", 9)],
            }
            tail_proj = [10, 11, 12]

            NB = int(os.environ.get("K_NB", str(BS)))
            if do_attn:
                prev = None
                for b in range(NB):
                    bigs = list(big_sched.get(b, [])) if do_proj or b < 3 else []
                    units = []  # attn units as closures
                    e2s = {}
                    Bt = bp.tile([64, 2, NTOK], f32, name="Bt")
                    for g in range(6):
                        e2 = e2p.tile([128, 2, 2, NTOK], bf16, name="e2")
                        e2s[g] = e2
                        units.append((emit_s, (b, g, 0, e2)))
                        units.append((emit_s, (b, g, 1, e2)))
                        if prev is not None:
                            units.append((emit_o, prev))
                        prev = (b, g, e2, Bt)
                    # interleave bigs evenly between attn units
                    nbig = len(bigs)
                    out_seq = []
                    bi = 0
                    for i, u in enumerate(units):
                        out_seq.append(u)
                        want = ((i + 1) * nbig) // len(units)
                        while bi < want:
                            out_seq.append(("big", bigs[bi]))
                            bi += 1
                    for u in out_seq:
                        if u[0] == "big":
                            kind = u[1]
                            if kind[0] == "v":
                                emit_v(kind[1], kind[2])
                            else:
                                emit_proj(kind[1])
                        else:
                            u[0](*u[1])
                if prev is not None:
                    emit_o(*prev)
                if do_proj:
                    for m in tail_proj:
                        emit_proj(m)

            if stop_after == "attn":
                nc.gpsimd.dma_start(out=y_d[0:128, :], in_=outT[:, 0, 0:DIM])
                for nm, rows in (("S", (128, 256)), ("e2", (256, 384)),
                                 ("O", (384, 512))):
                    nc.gpsimd.dma_start(
                        out=y_d[rows[0] : rows[1], 0 : 2 * NTOK].rearrange(
                            "p (s n) -> p s n", s=2
                        ),
                        in_=probes[nm][:, :, :],
                    )
                nc.gpsimd.dma_start(
                    out=y_d[512:576, 0 : 2 * NTOK].rearrange(
                        "p (s n) -> p s n", s=2
                    ),
                    in_=probes["Bt"][:, :, :],
                )

            for cm in (ps_y_cm, yp_cm, ps_o_cm, ps_s_cm, bp_cm, e2p_cm,
                       otp_cm, ps_v_cm, xp_cm):
                cm.__exit__(None, None, None)

    nc.compile()
    return nc


def _prep_inputs(x, qkv_w, q_bias, v_bias, rpb_table, proj_w, proj_b, rel_pos_index):
    bf16 = ml_dtypes.bfloat16
    x = np.asarray(x, np.float32)
    qkv_w = np.asarray(qkv_w, np.float32)
    q_bias = np.asarray(q_bias, np.float32)
    v_bias = np.asarray(v_bias, np.float32)
    rpb_table = np.asarray(rpb_table, np.float32)
    proj_w = np.asarray(proj_w, np.float32)
    proj_b = np.asarray(proj_b, np.float32)
    rel_pos_index = np.asarray(rel_pos_index)

    qkv_wt = qkv_w.T.copy()  # [768, 2304]
    qkv_wt[:, :DIM] *= SCALE
    qkv_wt = np.ascontiguousarray(qkv_wt, dtype=bf16)

    qb = (q_bias * SCALE).reshape(12, 64, 1).astype(np.float32)

    proj_wt = np.ascontiguousarray(proj_w.T, dtype=bf16)
    pb_eff = (proj_b + proj_w @ v_bias).reshape(1, DIM).astype(bf16)

    # bias[h, n, m] = rpb_table[rel_pos_index[n, m], h]; store raw (additive)
    # as [m-chunk, m-in-chunk, h*197 + n]
    bias_nmh = rpb_table[rel_pos_index]  # [n, m, h]
    er = bias_nmh.transpose(1, 2, 0)  # [m, h, n]
    er = er.reshape(NTOK, HEADS * NTOK)
    er_pad = np.zeros((256, HEADS * NTOK), np.float32)
    er_pad[:NTOK] = er
    rpb = np.ascontiguousarray(er_pad.reshape(2, 128, HEADS * NTOK), dtype=bf16)

    ident = np.eye(128, dtype=bf16)

    shared = {
        "qkv_wt": qkv_wt,
        "qb": qb,
        "proj_wt": proj_wt,
        "pb": pb_eff,
        "rpb": rpb,
        "ident": ident,
    }
    in_maps = []
    for c in range(NCORES):
        xc = x[c * BS : (c + 1) * BS].reshape(NT, DIM)
        xp = np.zeros((NTP, DIM), bf16)
        xp[:NT] = xc.astype(bf16)
        in_maps.append({"x": xp, **shared})
    return in_maps


def run(inputs, trace=False):
    """Build (cached), run on 8 cores, return (y_full, BassKernelResults)."""
    from concourse.bass_utils import run_bass_kernel_spmd

    if "nc" not in _CACHE:
        _CACHE["nc"] = _build_bass()
    nc = _CACHE["nc"]
    in_maps = _prep_inputs(**{k: inputs[k] for k in (
        "x", "qkv_w", "q_bias", "v_bias", "rpb_table", "proj_w", "proj_b",
        "rel_pos_index")})
    try:
        res = run_bass_kernel_spmd(
            nc, in_maps, core_ids=list(range(NCORES)), trace=trace
        )
    except ModuleNotFoundError:
        # NTFF profile hook unavailable in this container; run untraced
        res = run_bass_kernel_spmd(
            nc, in_maps, core_ids=list(range(NCORES)), trace=False
        )
    y = np.concatenate(
        [res.results[c]["y"].reshape(BS, NTOK, DIM) for c in range(NCORES)], axis=0
    )
    return np.ascontiguousarray(y, np.float32), res


def kernel(**inputs) -> np.ndarray:
    y, _ = run(inputs, trace=False)
    return y


# revision 36
# speedup vs baseline: 1.4123x; 1.0066x over previous
"""ViT attention block (B=64, N=197, H=12, hd=64, D=768) on 8 trn2 NeuronCores.

Pure data-parallel: 8 batches per core.  Single interleaved PE stream to keep
the HAM clock-gate warm (idle/low-utilization PE re-throttles to 1.2 GHz):

  prelude: q,k <- W_qk @ xT as 12 M=128 tiles (two heads per tile, k-outer
           loop over 4 live PSUM banks); v for all 8 batches.  xT is
           transposed on the HOST (no DMA-transpose, no xbar barrier).
  windows: attention for batch b interleaved with output-projection m-tiles
           (the N=512 proj matmuls keep the PE array streaming densely).

Attention per (batch b, head pair g, key chunk mc):
  S    = I @ rpb + k^T q      identity matmul (one N=394 strided-out MM)
                              preloads the relative-position bias into the
                              PSUM accumulation groups of both heads
  e2   = exp(S)               ACT, straight to bf16 SBUF (per-(b,g) tile)
  O    = [ones | v_h]^T @ e2  M=128: rows 0:64 = softmax sums (replicated),
                              rows 64:128 = unnormalized attention out
  B    = 1/O[0:64]            DVE reciprocal_approx_fast (input must sit at
                              partition base 0 - custom DVE ops ignore the
                              AP base_partition)
  outT = O[64:128] * B        DVE, head pairs stacked into partitions 0:64 /
                              64:128 of outT[128, 6, tok]
Projection: y = outT.T @ proj_wT (6 K=128 chunks) accumulated on top of a
K=1 ones-row matmul that preloads proj_b (v_bias folded in on host); evicted
by ACT copy, stored by gpsimd SWDGE.

Hardware constraints honored: PE operands and matmul PSUM outputs at
base_partition 0; one accumulation group per PSUM bank; gpsimd cannot access
PSUM; q pre-scaled by 1/8 on host.  DVE/ACT partition-base shifts verified
on HW.
"""

import os
import sys

import numpy as np

for _p in ("/opt/trn_rl_repo", os.path.expanduser("~/.axon_site/_ro/trn_rl_repo")):
    if os.path.isdir(_p) and _p not in sys.path:
        sys.path.insert(0, _p)

import ml_dtypes  # noqa: E402

B = 64
NTOK = 197
DIM = 768
HEADS = 12
HD = 64
NCORES = 8
BS = B // NCORES  # 8 batches per core
NT = BS * NTOK  # 1576 real tokens per core
NTP = 1600  # padded tokens (12x128 + 64)
SCALE = HD ** -0.5

_CACHE = {}


def _build_bass(stop_after=None):
    stop_after = stop_after or os.environ.get("K_STOP_AFTER", "")
    import concourse.mybir as mybir
    import concourse.tile as tile
    from concourse import bacc

    f32 = mybir.dt.float32
    bf16 = mybir.dt.bfloat16
    EXP = mybir.ActivationFunctionType.Exp

    nc = bacc.Bacc(
        "TRN2", target_bir_lowering=False, debug=False,
        num_devices=int(os.environ.get("K_NDEV", str(NCORES))),
    )

    xT_d = nc.dram_tensor("xT", [DIM, NTP], bf16, kind="ExternalInput")
    qkvw_d = nc.dram_tensor("qkv_wt", [DIM, 3 * DIM], bf16, kind="ExternalInput")
    qb_d = nc.dram_tensor("qb", [12, 64, 1], f32, kind="ExternalInput")
    projw_d = nc.dram_tensor("proj_wt", [DIM, DIM], bf16, kind="ExternalInput")
    pb_d = nc.dram_tensor("pb", [1, DIM], bf16, kind="ExternalInput")
    rpb_d = nc.dram_tensor("rpb", [2, 128, HEADS * NTOK], bf16, kind="ExternalInput")
    id_d = nc.dram_tensor("ident", [128, 128], bf16, kind="ExternalInput")
    y_d = nc.dram_tensor("y", [NT, DIM], f32, kind="ExternalOutput")

    NTILES = [(0, 512), (512, 512), (1024, 512), (1536, 64)]
    VTILES = [(0, 512), (512, 256)]

    with tile.TileContext(nc, linearize=bool(os.environ.get("K_LINEARIZE"))) as tc:
        with (
            tc.tile_pool(name="consts", bufs=1) as consts,
            tc.tile_pool(name="acts", bufs=1) as acts,
        ):
            # ---- constants ----
            projw = consts.tile([128, 6, DIM], bf16)  # head-pair K chunks
            rpb = consts.tile([128, 2, HEADS * NTOK], bf16)
            qb = consts.tile([64, 12, 1], f32)
            pb = consts.tile([1, DIM], bf16)
            ident = consts.tile([128, 128], bf16)
            ones1 = consts.tile([1, 128], bf16)

            # persistent activations
            qkT = acts.tile([64, 2 * HEADS, NTP], bf16)  # q heads 0-11, k 12-23
            vsb = acts.tile([128, 2 * BS, HEADS, 128], bf16)  # [ones | v_h]

            xp_cm = tc.tile_pool(name="xp", bufs=1)
            xp = xp_cm.__enter__()
            xt = xp.tile([128, 6, NTP], bf16)  # x transposed [c, tok]
            vw = xp.tile([128, 6, DIM], bf16)
            wqk_cm = tc.tile_pool(name="wqk", bufs=1)
            wqk = wqk_cm.__enter__()
            qkvw = wqk.tile([128, 6, 2 * DIM], bf16)

            qkvw_v = qkvw_d[:].rearrange("(k p) n -> p k n", p=128)
            xT_v = xT_d[:].rearrange("(k p) n -> p k n", p=128)
            for k in range(6):
                nc.sync.dma_start(out=qkvw[:, k, :], in_=qkvw_v[:, k, 0 : 2 * DIM])
                nc.sync.dma_start(out=xt[:, k, :], in_=xT_v[:, k, :])
            nc.sync.dma_start(out=qb[:, :, :], in_=qb_d[:].rearrange("k p o -> p k o"))
            for k in range(6):
                nc.sync.dma_start(out=vw[:, k, :], in_=qkvw_v[:, k, 2 * DIM : 3 * DIM])
            for mc in range(2):
                nc.sync.dma_start(out=rpb[:, mc, :], in_=rpb_d[mc, :, :])
            nc.sync.dma_start(out=pb[:, :], in_=pb_d[:, :])
            nc.sync.dma_start(out=ident[:, :], in_=id_d[:, :])
            projw_v = projw_d[:].rearrange("(k p) n -> p k n", p=128)
            for k in range(6):
                nc.sync.dma_start(out=projw[:, k, :], in_=projw_v[:, k, :])
            nc.vector.memset(vsb[:, :, :, 0:64], 1.0)
            nc.vector.memset(ones1[:, :], 1.0)

            do_qkv = stop_after != "load"
            do_attn = do_qkv and stop_after != "qkv"
            do_proj = do_attn and stop_after != "attn"

            ps_v_cm = tc.tile_pool(name="ps_v", bufs=2, space="PSUM")
            ps_v = ps_v_cm.__enter__()
            ps_qk_cm = tc.tile_pool(name="ps_qk", bufs=1, space="PSUM")
            ps_qk = ps_qk_cm.__enter__()

            def emit_qk(t):
                pss = [
                    ps_qk.tile([128, 512], f32, name=f"pss{j}") for j in range(4)
                ]
                for k in range(6):
                    for j, (noff, nsz) in enumerate(NTILES):
                        nc.tensor.matmul(
                            pss[j][:, :nsz],
                            qkvw[:, k, t * 128 : (t + 1) * 128],
                            xt[:, k, noff : noff + nsz],
                            start=(k == 0),
                            stop=(k == 5),
                        )
                for j, (noff, nsz) in enumerate(NTILES):
                    if t < 6:  # q: add bias (pre-scaled on host)
                        nc.vector.tensor_scalar_add(
                            qkT[:, 2 * t, noff : noff + nsz],
                            pss[j][0:64, :nsz],
                            qb[:, 2 * t, 0:1],
                        )
                        nc.vector.tensor_scalar_add(
                            qkT[:, 2 * t + 1, noff : noff + nsz],
                            pss[j][64:128, :nsz],
                            qb[:, 2 * t + 1, 0:1],
                        )
                    else:  # k: plain copies
                        h0 = 2 * (t - 6)
                        nc.scalar.copy(
                            qkT[:, HEADS + h0, noff : noff + nsz],
                            pss[j][0:64, :nsz],
                        )
                        nc.scalar.copy(
                            qkT[:, HEADS + h0 + 1, noff : noff + nsz],
                            pss[j][64:128, :nsz],
                        )

            def emit_v(b, mc):
                msz = 128 if mc == 0 else NTOK - 128
                toff = b * NTOK + mc * 128
                psv = ps_v.tile([128, DIM], f32, name="psv")
                for k in range(6):
                    for noff, nsz in VTILES:
                        nc.tensor.matmul(
                            psv[:msz, noff : noff + nsz],
                            xt[:, k, toff : toff + msz],
                            vw[:, k, noff : noff + nsz],
                            start=(k == 0),
                            stop=(k == 5),
                        )
                nc.scalar.copy(
                    vsb[:msz, b * 2 + mc, :, 64:128],
                    psv[:msz, :].rearrange("p (h d) -> p h d", d=64),
                )

            # ---- prelude: all of q,k and v ----
            for t in range(12 if do_qkv else 0):
                emit_qk(t)
            for b in range(BS if do_qkv else 0):
                for mc in range(2):
                    emit_v(b, mc)
            ps_qk_cm.__exit__(None, None, None)
            ps_v_cm.__exit__(None, None, None)
            wqk_cm.__exit__(None, None, None)
            xp_cm.__exit__(None, None, None)

            otp_cm = tc.tile_pool(name="otp", bufs=1)
            otp = otp_cm.__enter__()
            outT = otp.tile([128, 6, NTP], bf16)  # attn out, head pairs stacked

            if stop_after == "qkv":
                nc.gpsimd.dma_start(out=y_d[0:64, :], in_=qkT[:, 0, 0:DIM])
                nc.gpsimd.dma_start(out=y_d[64:128, :], in_=qkT[:, 1, 0:DIM])
                nc.gpsimd.dma_start(
                    out=y_d[128:256, :].rearrange("p (h d) -> p h d", d=64),
                    in_=vsb[:, 0, :, 64:128],
                )
                nc.gpsimd.dma_start(out=y_d[256:320, :], in_=qkT[:, 12, 0:DIM])
                nc.gpsimd.dma_start(out=y_d[320:384, :], in_=qkT[:, 13, 0:DIM])
                nc.gpsimd.dma_start(
                    out=y_d[384:453, :].rearrange("p (h d) -> p h d", d=64),
                    in_=vsb[0:69, 1, :, 64:128],
                )

            # ---- attention interleaved with projection ----
            probes = {}
            if stop_after == "attn":
                probes["S"] = otp.tile([128, 2, NTOK], f32, name="probe_S")
                probes["e2"] = otp.tile([128, 2, NTOK], f32, name="probe_e2")
                probes["O"] = otp.tile([128, 2, NTOK], f32, name="probe_O")
                probes["Bt"] = otp.tile([64, 2, NTOK], f32, name="probe_Bt")

            e2p_cm = tc.tile_pool(name="e2p", bufs=4)
            e2p = e2p_cm.__enter__()
            bp_cm = tc.tile_pool(name="bp", bufs=2)
            bp = bp_cm.__enter__()
            ps_s_cm = tc.tile_pool(name="ps_s", bufs=2, space="PSUM")
            ps_s = ps_s_cm.__enter__()
            ps_o_cm = tc.tile_pool(name="ps_o", bufs=1, space="PSUM")
            ps_o = ps_o_cm.__enter__()
            yp_cm = tc.tile_pool(name="yp", bufs=2)
            yp = yp_cm.__enter__()
            ps_y_cm = tc.tile_pool(name="ps_y", bufs=1, space="PSUM")
            ps_y = ps_y_cm.__enter__()

            def emit_s(b, g, mc, e2):
                tb = b * NTOK
                msz = 128 if mc == 0 else NTOK - 128
                S = ps_s.tile([128, 1024], f32, name="S")
                Sv = S[:msz, :].rearrange("p (s n) -> p s n", s=2)[:, :, :NTOK]
                for hh in range(2):
                    h = 2 * g + hh
                    nc.tensor.matmul(
                        S[:msz, hh * 512 : hh * 512 + NTOK],
                        ident[:msz, :msz],
                        rpb[:msz, mc, h * NTOK : (h + 1) * NTOK],
                        start=True,
                        stop=False,
                    )
                    nc.tensor.matmul(
                        S[:msz, hh * 512 : hh * 512 + NTOK],
                        qkT[:, HEADS + h, tb + mc * 128 : tb + mc * 128 + msz],
                        qkT[:, h, tb : tb + NTOK],
                        start=False,
                        stop=True,
                    )
                nc.scalar.activation(e2[:msz, mc, :, :], Sv, EXP)
                if probes and b == 0 and g == 0 and mc == 0:
                    nc.vector.tensor_copy(
                        probes["S"][:, :, :],
                        S[:, :].rearrange("p (s n) -> p s n", s=2)[:, :, :NTOK],
                    )
                    nc.scalar.copy(probes["e2"][:, :, :], e2[:, 0, :, :])

            def emit_o(b, g, e2, Bt):
                tb = b * NTOK
                O = ps_o.tile([128, 2, 512], f32, name="O")
                for hh in range(2):
                    h = 2 * g + hh
                    for mc in range(2):
                        msz = 128 if mc == 0 else NTOK - 128
                        nc.tensor.matmul(
                            O[:, hh, 0:NTOK],
                            vsb[:msz, b * 2 + mc, h, :],
                            e2[:msz, mc, hh, :],
                            start=(mc == 0),
                            stop=(mc == 1),
                        )
                if probes and b == 0 and g == 0:
                    nc.vector.tensor_copy(probes["O"][:, :, :], O[:, :, 0:NTOK])
                nc.vector.reciprocal_approx_fast(
                    out=Bt[:, :, :], in_=O[0:64, :, 0:NTOK]
                )
                if probes and b == 0 and g == 0:
                    nc.vector.tensor_copy(probes["Bt"][:, :, :], Bt[:, :, :])
                for hh in range(2):
                    nc.vector.tensor_mul(
                        outT[hh * 64 : (hh + 1) * 64, g, tb : tb + NTOK],
                        O[64:128, hh, 0:NTOK],
                        Bt[:, hh, :],
                    )

            def emit_proj(m):
                moff = m * 128
                msz = min(128, NTP - moff)
                real = min(128, NT - moff)
                Y = ps_y.tile([128, DIM], f32, name="Y")
                for noff, nsz in VTILES:
                    nc.tensor.matmul(
                        Y[:msz, noff : noff + nsz],
                        ones1[0:1, 0:msz],
                        pb[0:1, noff : noff + nsz],
                        start=True,
                        stop=False,
                    )
                for kp in range(6):
                    for noff, nsz in VTILES:
                        nc.tensor.matmul(
                            Y[:msz, noff : noff + nsz],
                            outT[:, kp, moff : moff + msz],
                            projw[:, kp, noff : noff + nsz],
                            start=False,
                            stop=(kp == 5),
                        )
                ysb = yp.tile([128, DIM], f32, name="ysb")
                nc.scalar.copy(ysb[:msz, :], Y[:msz, :])
                nc.gpsimd.dma_start(
                    out=y_d[moff : moff + real, :], in_=ysb[:real, :]
                )

            # proj m-tile needs batches <= (128m+127)//197 fully emitted; a
            # batch's last O unit lands at position ~3 of the NEXT window.
            proj_sched = {1: [0], 2: [1], 3: [2, 3], 4: [4, 5], 5: [6],
                          6: [7, 8], 7: [9]}
            tail_proj = [10, 11, 12]

            NB = int(os.environ.get("K_NB", str(BS)))
            if do_attn:
                prev = None
                for b in range(NB):
                    bigs = list(proj_sched.get(b, [])) if do_proj else []
                    units = []
                    Bt = bp.tile([64, 2, NTOK], f32, name="Bt")
                    for g in range(6):
                        e2 = e2p.tile([128, 2, 2, NTOK], bf16, name="e2")
                        units.append((emit_s, (b, g, 0, e2)))
                        units.append((emit_s, (b, g, 1, e2)))
                        if prev is not None:
                            units.append((emit_o, prev))
                        prev = (b, g, e2, Bt)
                    # spread proj units across positions 5..len(units)
                    nbig = len(bigs)
                    pos = {
                        4 + (j + 1) * (len(units) - 4) // (nbig + 1): bigs[j]
                        for j in range(nbig)
                    }
                    for i, u in enumerate(units):
                        u[0](*u[1])
                        if i + 1 in pos:
                            emit_proj(pos[i + 1])
                if prev is not None:
                    emit_o(*prev)
                if do_proj:
                    for m in tail_proj:
                        emit_proj(m)

            if stop_after == "attn":
                nc.gpsimd.dma_start(out=y_d[0:128, :], in_=outT[:, 0, 0:DIM])
                for nm, rows in (("S", (128, 256)), ("e2", (256, 384)),
                                 ("O", (384, 512))):
                    nc.gpsimd.dma_start(
                        out=y_d[rows[0] : rows[1], 0 : 2 * NTOK].rearrange(
                            "p (s n) -> p s n", s=2
                        ),
                        in_=probes[nm][:, :, :],
                    )
                nc.gpsimd.dma_start(
                    out=y_d[512:576, 0 : 2 * NTOK].rearrange(
                        "p (s n) -> p s n", s=2
                    ),
                    in_=probes["Bt"][:, :, :],
                )

            for cm in (ps_y_cm, yp_cm, ps_o_cm, ps_s_cm, bp_cm, e2p_cm,
                       otp_cm):
                cm.__exit__(None, None, None)

    nc.compile()
    return nc


def _prep_inputs(x, qkv_w, q_bias, v_bias, rpb_table, proj_w, proj_b, rel_pos_index):
    bf16 = ml_dtypes.bfloat16
    x = np.asarray(x, np.float32)
    qkv_w = np.asarray(qkv_w, np.float32)
    q_bias = np.asarray(q_bias, np.float32)
    v_bias = np.asarray(v_bias, np.float32)
    rpb_table = np.asarray(rpb_table, np.float32)
    proj_w = np.asarray(proj_w, np.float32)
    proj_b = np.asarray(proj_b, np.float32)
    rel_pos_index = np.asarray(rel_pos_index)

    qkv_wt = qkv_w.T.copy()  # [768, 2304]
    qkv_wt[:, :DIM] *= SCALE
    qkv_wt = np.ascontiguousarray(qkv_wt, dtype=bf16)

    qb = (q_bias * SCALE).reshape(12, 64, 1).astype(np.float32)

    proj_wt = np.ascontiguousarray(proj_w.T, dtype=bf16)
    pb_eff = (proj_b + proj_w @ v_bias).reshape(1, DIM).astype(bf16)

    # bias[h, n, m] = rpb_table[rel_pos_index[n, m], h]; store raw (additive)
    # as [m-chunk, m-in-chunk, h*197 + n]
    bias_nmh = rpb_table[rel_pos_index]  # [n, m, h]
    er = bias_nmh.transpose(1, 2, 0)  # [m, h, n]
    er = er.reshape(NTOK, HEADS * NTOK)
    er_pad = np.zeros((256, HEADS * NTOK), np.float32)
    er_pad[:NTOK] = er
    rpb = np.ascontiguousarray(er_pad.reshape(2, 128, HEADS * NTOK), dtype=bf16)

    ident = np.eye(128, dtype=bf16)

    shared = {
        "qkv_wt": qkv_wt,
        "qb": qb,
        "proj_wt": proj_wt,
        "pb": pb_eff,
        "rpb": rpb,
        "ident": ident,
    }
    in_maps = []
    for c in range(NCORES):
        xc = x[c * BS : (c + 1) * BS].reshape(NT, DIM)
        xp = np.zeros((NTP, DIM), np.float32)
        xp[:NT] = xc
        xT = np.ascontiguousarray(xp.T, dtype=bf16)  # [768, 1600]
        in_maps.append({"xT": xT, **shared})
    return in_maps


def run(inputs, trace=False):
    """Build (cached), run on 8 cores, return (y_full, BassKernelResults)."""
    from concourse.bass_utils import run_bass_kernel_spmd

    if "nc" not in _CACHE:
        _CACHE["nc"] = _build_bass()
    nc = _CACHE["nc"]
    in_maps = _prep_inputs(**{k: inputs[k] for k in (
        "x", "qkv_w", "q_bias", "v_bias", "rpb_table", "proj_w", "proj_b",
        "rel_pos_index")})
    try:
        res = run_bass_kernel_spmd(
            nc, in_maps, core_ids=list(range(NCORES)), trace=trace
        )
    except ModuleNotFoundError:
        # NTFF profile hook unavailable in this container; run untraced
        res = run_bass_kernel_spmd(
            nc, in_maps, core_ids=list(range(NCORES)), trace=False
        )
    y = np.concatenate(
        [res.results[c]["y"].reshape(BS, NTOK, DIM) for c in range(NCORES)], axis=0
    )
    return np.ascontiguousarray(y, np.float32), res


def kernel(**inputs) -> np.ndarray:
    y, _ = run(inputs, trace=False)
    return y


# revision 42
# speedup vs baseline: 1.6857x; 1.1936x over previous
"""ViT attention block (B=64, N=197, H=12, hd=64, D=768) on 8 trn2 NeuronCores.

Pure data-parallel: 8 batches per core.  Single interleaved PE stream to keep
the HAM clock-gate warm (idle/low-utilization PE re-throttles to 1.2 GHz):

  prelude: q,k <- W_qk @ xT as 12 M=128 tiles (two heads per tile, k-outer
           loop over 4 live PSUM banks); v for all 8 batches.  xT is
           transposed on the HOST (no DMA-transpose, no xbar barrier).
  windows: attention for batch b interleaved with output-projection m-tiles
           (the N=512 proj matmuls keep the PE array streaming densely).

Attention per (batch b, head pair g, key chunk mc):
  S    = I @ rpb + k^T q      identity matmul (one N=394 strided-out MM)
                              preloads the relative-position bias into the
                              PSUM accumulation groups of both heads
  e2   = exp(S)               ACT, straight to bf16 SBUF (per-(b,g) tile)
  O    = [ones | v_h]^T @ e2  M=128: rows 0:64 = softmax sums (replicated),
                              rows 64:128 = unnormalized attention out
  B    = 1/O[0:64]            DVE reciprocal_approx_fast (input must sit at
                              partition base 0 - custom DVE ops ignore the
                              AP base_partition)
  outT = O[64:128] * B        DVE, head pairs stacked into partitions 0:64 /
                              64:128 of outT[128, 6, tok]
Projection: y = outT.T @ proj_wT (6 K=128 chunks) accumulated on top of a
K=1 ones-row matmul that preloads proj_b (v_bias folded in on host); evicted
by ACT copy, stored by gpsimd SWDGE.

Hardware constraints honored: PE operands and matmul PSUM outputs at
base_partition 0; one accumulation group per PSUM bank; gpsimd cannot access
PSUM; q pre-scaled by 1/8 on host.  DVE/ACT partition-base shifts verified
on HW.
"""

import os
import sys

import numpy as np

for _p in ("/opt/trn_rl_repo", os.path.expanduser("~/.axon_site/_ro/trn_rl_repo")):
    if os.path.isdir(_p) and _p not in sys.path:
        sys.path.insert(0, _p)

import ml_dtypes  # noqa: E402

B = 64
NTOK = 197
DIM = 768
HEADS = 12
HD = 64
NCORES = 8
BS = B // NCORES  # 8 batches per core
NT = BS * NTOK  # 1576 real tokens per core
NTP = 1600  # padded tokens (12x128 + 64)
SCALE = HD ** -0.5

_CACHE = {}


def _build_bass(stop_after=None):
    stop_after = stop_after or os.environ.get("K_STOP_AFTER", "")
    import concourse.mybir as mybir
    import concourse.tile as tile
    from concourse import bacc

    f32 = mybir.dt.float32
    bf16 = mybir.dt.bfloat16
    EXP = mybir.ActivationFunctionType.Exp

    nc = bacc.Bacc(
        "TRN2", target_bir_lowering=False, debug=False,
        num_devices=int(os.environ.get("K_NDEV", str(NCORES))),
    )

    xT_d = nc.dram_tensor("xT", [DIM, NTP], bf16, kind="ExternalInput")
    qkvw_d = nc.dram_tensor("qkv_wt", [DIM, 3 * DIM], bf16, kind="ExternalInput")
    qb_d = nc.dram_tensor("qb", [12, 64, 1], f32, kind="ExternalInput")
    projw_d = nc.dram_tensor("proj_wt", [DIM, DIM], bf16, kind="ExternalInput")
    pb_d = nc.dram_tensor("pb", [1, DIM], bf16, kind="ExternalInput")
    rpb_d = nc.dram_tensor("rpb", [2, 128, HEADS * NTOK], bf16, kind="ExternalInput")
    y_d = nc.dram_tensor("y", [NT, DIM], f32, kind="ExternalOutput")

    NTILES = [(0, 512), (512, 512), (1024, 512), (1536, 64)]
    VTILES = [(0, 512), (512, 256)]

    with tile.TileContext(nc, linearize=bool(os.environ.get("K_LINEARIZE"))) as tc:
        with (
            tc.tile_pool(name="consts", bufs=1) as consts,
            tc.tile_pool(name="acts", bufs=1) as acts,
        ):
            # ---- constants ----
            projw = consts.tile([128, 6, DIM], bf16)  # head-pair K chunks
            rpb = consts.tile([128, 2, HEADS * NTOK], bf16)
            qb = consts.tile([64, 12, 1], f32)
            pb = consts.tile([1, DIM], bf16)
            ones1 = consts.tile([1, 128], bf16)

            # persistent activations
            qkT = acts.tile([64, 2 * HEADS, NTP], bf16)  # q heads 0-11, k 12-23
            vsb = acts.tile([128, 2 * BS, HEADS, 128], bf16)  # [ones | v_h]

            xp_cm = tc.tile_pool(name="xp", bufs=1)
            xp = xp_cm.__enter__()
            xt = xp.tile([128, 6, NTP], bf16)  # x transposed [c, tok]
            vw = xp.tile([128, 6, DIM], bf16)
            wqk_cm = tc.tile_pool(name="wqk", bufs=1)
            wqk = wqk_cm.__enter__()
            qkvw = wqk.tile([128, 6, 2 * DIM], bf16)

            qkvw_v = qkvw_d[:].rearrange("(k p) n -> p k n", p=128)
            xT_v = xT_d[:].rearrange("(k p) n -> p k n", p=128)
            for k in range(6):
                nc.sync.dma_start(out=qkvw[:, k, :], in_=qkvw_v[:, k, 0 : 2 * DIM])
                nc.sync.dma_start(out=xt[:, k, :], in_=xT_v[:, k, :])
            nc.sync.dma_start(out=qb[:, :, :], in_=qb_d[:].rearrange("k p o -> p k o"))
            for k in range(6):
                nc.sync.dma_start(out=vw[:, k, :], in_=qkvw_v[:, k, 2 * DIM : 3 * DIM])
            for mc in range(2):
                nc.sync.dma_start(out=rpb[:, mc, :], in_=rpb_d[mc, :, :])
            nc.sync.dma_start(out=pb[:, :], in_=pb_d[:, :])
            projw_v = projw_d[:].rearrange("(k p) n -> p k n", p=128)
            for k in range(6):
                nc.sync.dma_start(out=projw[:, k, :], in_=projw_v[:, k, :])
            nc.vector.memset(vsb[:, :, :, 0:64], 1.0)
            nc.vector.memset(ones1[:, :], 1.0)

            do_qkv = stop_after != "load"
            do_attn = do_qkv and stop_after != "qkv"
            do_proj = do_attn and stop_after != "attn"

            ps_v_cm = tc.tile_pool(name="ps_v", bufs=2, space="PSUM")
            ps_v = ps_v_cm.__enter__()
            ps_qk_cm = tc.tile_pool(name="ps_qk", bufs=1, space="PSUM")
            ps_qk = ps_qk_cm.__enter__()

            def emit_qk(t):
                pss = [
                    ps_qk.tile([128, 512], f32, name=f"pss{j}") for j in range(4)
                ]
                for k in range(6):
                    for j, (noff, nsz) in enumerate(NTILES):
                        nc.tensor.matmul(
                            pss[j][:, :nsz],
                            qkvw[:, k, t * 128 : (t + 1) * 128],
                            xt[:, k, noff : noff + nsz],
                            start=(k == 0),
                            stop=(k == 5),
                        )
                for j, (noff, nsz) in enumerate(NTILES):
                    if t < 6:  # q: add bias (pre-scaled on host)
                        nc.vector.tensor_scalar_add(
                            qkT[:, 2 * t, noff : noff + nsz],
                            pss[j][0:64, :nsz],
                            qb[:, 2 * t, 0:1],
                        )
                        nc.vector.tensor_scalar_add(
                            qkT[:, 2 * t + 1, noff : noff + nsz],
                            pss[j][64:128, :nsz],
                            qb[:, 2 * t + 1, 0:1],
                        )
                    else:  # k: plain copies
                        h0 = 2 * (t - 6)
                        nc.scalar.copy(
                            qkT[:, HEADS + h0, noff : noff + nsz],
                            pss[j][0:64, :nsz],
                        )
                        nc.scalar.copy(
                            qkT[:, HEADS + h0 + 1, noff : noff + nsz],
                            pss[j][64:128, :nsz],
                        )

            def emit_v(b, mc):
                msz = 128 if mc == 0 else NTOK - 128
                toff = b * NTOK + mc * 128
                psv = ps_v.tile([128, DIM], f32, name="psv")
                for k in range(6):
                    for noff, nsz in VTILES:
                        nc.tensor.matmul(
                            psv[:msz, noff : noff + nsz],
                            xt[:, k, toff : toff + msz],
                            vw[:, k, noff : noff + nsz],
                            start=(k == 0),
                            stop=(k == 5),
                        )
                nc.scalar.copy(
                    vsb[:msz, b * 2 + mc, :, 64:128],
                    psv[:msz, :].rearrange("p (h d) -> p h d", d=64),
                )

            # ---- prelude: all of q,k and v ----
            for t in range(12 if do_qkv else 0):
                emit_qk(t)
            for b in range(BS if do_qkv else 0):
                for mc in range(2):
                    emit_v(b, mc)
            ps_qk_cm.__exit__(None, None, None)
            ps_v_cm.__exit__(None, None, None)
            wqk_cm.__exit__(None, None, None)
            xp_cm.__exit__(None, None, None)

            otp_cm = tc.tile_pool(name="otp", bufs=1)
            otp = otp_cm.__enter__()
            outT = otp.tile([128, 6, NTP], bf16)  # attn out, head pairs stacked

            if stop_after == "qkv":
                nc.gpsimd.dma_start(out=y_d[0:64, :], in_=qkT[:, 0, 0:DIM])
                nc.gpsimd.dma_start(out=y_d[64:128, :], in_=qkT[:, 1, 0:DIM])
                nc.gpsimd.dma_start(
                    out=y_d[128:256, :].rearrange("p (h d) -> p h d", d=64),
                    in_=vsb[:, 0, :, 64:128],
                )
                nc.gpsimd.dma_start(out=y_d[256:320, :], in_=qkT[:, 12, 0:DIM])
                nc.gpsimd.dma_start(out=y_d[320:384, :], in_=qkT[:, 13, 0:DIM])
                nc.gpsimd.dma_start(
                    out=y_d[384:453, :].rearrange("p (h d) -> p h d", d=64),
                    in_=vsb[0:69, 1, :, 64:128],
                )

            # ---- attention interleaved with projection ----
            probes = {}
            if stop_after == "attn":
                probes["S"] = otp.tile([128, 2, NTOK], f32, name="probe_S")
                probes["e2"] = otp.tile([128, 2, NTOK], f32, name="probe_e2")
                probes["O"] = otp.tile([128, 2, NTOK], f32, name="probe_O")
                probes["Bt"] = otp.tile([64, 2, NTOK], f32, name="probe_Bt")

            e2p_cm = tc.tile_pool(name="e2p", bufs=4)
            e2p = e2p_cm.__enter__()
            bp_cm = tc.tile_pool(name="bp", bufs=2)
            bp = bp_cm.__enter__()
            ps_s_cm = tc.tile_pool(name="ps_s", bufs=2, space="PSUM")
            ps_s = ps_s_cm.__enter__()
            ps_o_cm = tc.tile_pool(name="ps_o", bufs=1, space="PSUM")
            ps_o = ps_o_cm.__enter__()
            yp_cm = tc.tile_pool(name="yp", bufs=2)
            yp = yp_cm.__enter__()
            ps_y_cm = tc.tile_pool(name="ps_y", bufs=1, space="PSUM")
            ps_y = ps_y_cm.__enter__()

            def emit_s(b, g, mc, e2):
                tb = b * NTOK
                msz = 128 if mc == 0 else NTOK - 128
                S = ps_s.tile([128, 1024], f32, name="S")
                Sv = S[:msz, :].rearrange("p (s n) -> p s n", s=2)[:, :, :NTOK]
                for hh in range(2):
                    h = 2 * g + hh
                    nc.tensor.matmul(
                        S[:msz, hh * 512 : hh * 512 + NTOK],
                        qkT[:, HEADS + h, tb + mc * 128 : tb + mc * 128 + msz],
                        qkT[:, h, tb : tb + NTOK],
                        start=True,
                        stop=True,
                    )
                nc.scalar.activation(e2[:msz, mc, :, :], Sv, EXP)
                # exp(rpb) multiplied in on the otherwise-idle gpsimd engine
                nc.gpsimd.tensor_mul(
                    e2[:msz, mc, :, :],
                    e2[:msz, mc, :, :],
                    rpb[:msz, mc, 2 * g * NTOK : (2 * g + 2) * NTOK].rearrange(
                        "p (s n) -> p s n", s=2
                    ),
                )
                if probes and b == 0 and g == 0 and mc == 0:
                    nc.vector.tensor_copy(
                        probes["S"][:, :, :],
                        S[:, :].rearrange("p (s n) -> p s n", s=2)[:, :, :NTOK],
                    )
                    nc.scalar.copy(probes["e2"][:, :, :], e2[:, 0, :, :])

            def emit_o(b, g, e2, Bt):
                tb = b * NTOK
                O = ps_o.tile([128, 2, 512], f32, name="O")
                for hh in range(2):
                    h = 2 * g + hh
                    for mc in range(2):
                        msz = 128 if mc == 0 else NTOK - 128
                        nc.tensor.matmul(
                            O[:, hh, 0:NTOK],
                            vsb[:msz, b * 2 + mc, h, :],
                            e2[:msz, mc, hh, :],
                            start=(mc == 0),
                            stop=(mc == 1),
                        )
                if probes and b == 0 and g == 0:
                    nc.vector.tensor_copy(probes["O"][:, :, :], O[:, :, 0:NTOK])
                nc.vector.reciprocal_approx_fast(
                    out=Bt[:, :, :], in_=O[0:64, :, 0:NTOK]
                )
                if probes and b == 0 and g == 0:
                    nc.vector.tensor_copy(probes["Bt"][:, :, :], Bt[:, :, :])
                for hh in range(2):
                    nc.vector.tensor_mul(
                        outT[hh * 64 : (hh + 1) * 64, g, tb : tb + NTOK],
                        O[64:128, hh, 0:NTOK],
                        Bt[:, hh, :],
                    )

            def emit_proj(m):
                moff = m * 128
                msz = min(128, NTP - moff)
                real = min(128, NT - moff)
                Y = ps_y.tile([128, DIM], f32, name="Y")
                for noff, nsz in VTILES:
                    nc.tensor.matmul(
                        Y[:msz, noff : noff + nsz],
                        ones1[0:1, 0:msz],
                        pb[0:1, noff : noff + nsz],
                        start=True,
                        stop=False,
                    )
                for kp in range(6):
                    for noff, nsz in VTILES:
                        nc.tensor.matmul(
                            Y[:msz, noff : noff + nsz],
                            outT[:, kp, moff : moff + msz],
                            projw[:, kp, noff : noff + nsz],
                            start=False,
                            stop=(kp == 5),
                        )
                ysb = yp.tile([128, DIM], f32, name="ysb")
                nc.scalar.copy(ysb[:msz, :], Y[:msz, :])
                nc.gpsimd.dma_start(
                    out=y_d[moff : moff + real, :], in_=ysb[:real, :]
                )

            # proj m-tile needs batches <= (128m+127)//197 fully emitted; a
            # batch's last O unit lands at position ~3 of the NEXT window.
            proj_sched = {1: [0], 2: [1], 3: [2, 3], 4: [4, 5], 5: [6],
                          6: [7, 8], 7: [9]}
            tail_proj = [10, 11, 12]

            NB = int(os.environ.get("K_NB", str(BS)))
            OLAG = int(os.environ.get("K_OLAG", "2"))
            if do_attn:
                pending = []
                for b in range(NB):
                    bigs = list(proj_sched.get(b, [])) if do_proj else []
                    units = []
                    Bt = bp.tile([64, 2, NTOK], f32, name="Bt")
                    for g in range(6):
                        e2 = e2p.tile([128, 2, 2, NTOK], bf16, name="e2")
                        units.append((emit_s, (b, g, 0, e2)))
                        units.append((emit_s, (b, g, 1, e2)))
                        pending.append((b, g, e2, Bt))
                        if len(pending) > OLAG:
                            units.append((emit_o, pending.pop(0)))
                    # spread proj units across positions 5..len(units)
                    nbig = len(bigs)
                    pos = {
                        4 + (j + 1) * (len(units) - 4) // (nbig + 1): bigs[j]
                        for j in range(nbig)
                    }
                    for i, u in enumerate(units):
                        u[0](*u[1])
                        if i + 1 in pos:
                            emit_proj(pos[i + 1])
                for p in pending:
                    emit_o(p[0], p[1], p[2], p[3])
                if do_proj:
                    for m in tail_proj:
                        emit_proj(m)

            if stop_after == "attn":
                nc.gpsimd.dma_start(out=y_d[0:128, :], in_=outT[:, 0, 0:DIM])
                for nm, rows in (("S", (128, 256)), ("e2", (256, 384)),
                                 ("O", (384, 512))):
                    nc.gpsimd.dma_start(
                        out=y_d[rows[0] : rows[1], 0 : 2 * NTOK].rearrange(
                            "p (s n) -> p s n", s=2
                        ),
                        in_=probes[nm][:, :, :],
                    )
                nc.gpsimd.dma_start(
                    out=y_d[512:576, 0 : 2 * NTOK].rearrange(
                        "p (s n) -> p s n", s=2
                    ),
                    in_=probes["Bt"][:, :, :],
                )

            for cm in (ps_y_cm, yp_cm, ps_o_cm, ps_s_cm, bp_cm, e2p_cm,
                       otp_cm):
                cm.__exit__(None, None, None)

    nc.compile()
    return nc


def _prep_inputs(x, qkv_w, q_bias, v_bias, rpb_table, proj_w, proj_b, rel_pos_index):
    bf16 = ml_dtypes.bfloat16
    x = np.asarray(x, np.float32)
    qkv_w = np.asarray(qkv_w, np.float32)
    q_bias = np.asarray(q_bias, np.float32)
    v_bias = np.asarray(v_bias, np.float32)
    rpb_table = np.asarray(rpb_table, np.float32)
    proj_w = np.asarray(proj_w, np.float32)
    proj_b = np.asarray(proj_b, np.float32)
    rel_pos_index = np.asarray(rel_pos_index)

    qkv_wt = qkv_w.T.copy()  # [768, 2304]
    qkv_wt[:, :DIM] *= SCALE
    qkv_wt = np.ascontiguousarray(qkv_wt, dtype=bf16)

    qb = (q_bias * SCALE).reshape(12, 64, 1).astype(np.float32)

    proj_wt = np.ascontiguousarray(proj_w.T, dtype=bf16)
    pb_eff = (proj_b + proj_w @ v_bias).reshape(1, DIM).astype(bf16)

    # bias[h, n, m] = rpb_table[rel_pos_index[n, m], h]; store exp()
    # (multiplicative) as [m-chunk, m-in-chunk, h*197 + n]
    bias_nmh = rpb_table[rel_pos_index]  # [n, m, h]
    er = np.exp(bias_nmh.transpose(1, 2, 0))  # [m, h, n]
    er = er.reshape(NTOK, HEADS * NTOK)
    er_pad = np.ones((256, HEADS * NTOK), np.float32)
    er_pad[:NTOK] = er
    rpb = np.ascontiguousarray(er_pad.reshape(2, 128, HEADS * NTOK), dtype=bf16)

    shared = {
        "qkv_wt": qkv_wt,
        "qb": qb,
        "proj_wt": proj_wt,
        "pb": pb_eff,
        "rpb": rpb,
    }
    in_maps = []
    for c in range(NCORES):
        xc = x[c * BS : (c + 1) * BS].reshape(NT, DIM)
        xp = np.zeros((NTP, DIM), np.float32)
        xp[:NT] = xc
        xT = np.ascontiguousarray(xp.T, dtype=bf16)  # [768, 1600]
        in_maps.append({"xT": xT, **shared})
    return in_maps


def run(inputs, trace=False):
    """Build (cached), run on 8 cores, return (y_full, BassKernelResults)."""
    from concourse.bass_utils import run_bass_kernel_spmd

    if "nc" not in _CACHE:
        _CACHE["nc"] = _build_bass()
    nc = _CACHE["nc"]
    in_maps = _prep_inputs(**{k: inputs[k] for k in (
        "x", "qkv_w", "q_bias", "v_bias", "rpb_table", "proj_w", "proj_b",
        "rel_pos_index")})
    try:
        res = run_bass_kernel_spmd(
            nc, in_maps, core_ids=list(range(NCORES)), trace=trace
        )
    except ModuleNotFoundError:
        # NTFF profile hook unavailable in this container; run untraced
        res = run_bass_kernel_spmd(
            nc, in_maps, core_ids=list(range(NCORES)), trace=False
        )
    y = np.concatenate(
        [res.results[c]["y"].reshape(BS, NTOK, DIM) for c in range(NCORES)], axis=0
    )
    return np.ascontiguousarray(y, np.float32), res


def kernel(**inputs) -> np.ndarray:
    y, _ = run(inputs, trace=False)
    return y


# revision 46
# speedup vs baseline: 1.7096x; 1.0142x over previous
"""ViT attention block (B=64, N=197, H=12, hd=64, D=768) on 8 trn2 NeuronCores.

Pure data-parallel: 8 batches per core.  Single interleaved PE stream to keep
the HAM clock-gate warm (idle/low-utilization PE re-throttles to 1.2 GHz):

  prelude: q,k <- W_qk @ xT as 12 M=128 tiles (two heads per tile, k-outer
           loop over 4 live PSUM banks); v for all 8 batches.  xT is
           transposed on the HOST (no DMA-transpose, no xbar barrier).
  windows: attention for batch b interleaved with output-projection m-tiles
           (the N=512 proj matmuls keep the PE array streaming densely).

Attention per (batch b, head pair g, key chunk mc):
  S    = I @ rpb + k^T q      identity matmul (one N=394 strided-out MM)
                              preloads the relative-position bias into the
                              PSUM accumulation groups of both heads
  e2   = exp(S)               ACT, straight to bf16 SBUF (per-(b,g) tile)
  O    = [ones | v_h]^T @ e2  M=128: rows 0:64 = softmax sums (replicated),
                              rows 64:128 = unnormalized attention out
  B    = 1/O[0:64]            DVE reciprocal_approx_fast (input must sit at
                              partition base 0 - custom DVE ops ignore the
                              AP base_partition)
  outT = O[64:128] * B        DVE, head pairs stacked into partitions 0:64 /
                              64:128 of outT[128, 6, tok]
Projection: y = outT.T @ proj_wT (6 K=128 chunks) accumulated on top of a
K=1 ones-row matmul that preloads proj_b (v_bias folded in on host); evicted
by ACT copy, stored by gpsimd SWDGE.

Hardware constraints honored: PE operands and matmul PSUM outputs at
base_partition 0; one accumulation group per PSUM bank; gpsimd cannot access
PSUM; q pre-scaled by 1/8 on host.  DVE/ACT partition-base shifts verified
on HW.
"""

import os
import sys

import numpy as np

for _p in ("/opt/trn_rl_repo", os.path.expanduser("~/.axon_site/_ro/trn_rl_repo")):
    if os.path.isdir(_p) and _p not in sys.path:
        sys.path.insert(0, _p)

import ml_dtypes  # noqa: E402

B = 64
NTOK = 197
DIM = 768
HEADS = 12
HD = 64
NCORES = 8
BS = B // NCORES  # 8 batches per core
NT = BS * NTOK  # 1576 real tokens per core
NTP = 1600  # padded tokens (12x128 + 64)
SCALE = HD ** -0.5

_CACHE = {}


def _build_bass(stop_after=None):
    stop_after = stop_after or os.environ.get("K_STOP_AFTER", "")
    import concourse.mybir as mybir
    import concourse.tile as tile
    from concourse import bacc

    f32 = mybir.dt.float32
    bf16 = mybir.dt.bfloat16
    EXP = mybir.ActivationFunctionType.Exp

    nc = bacc.Bacc(
        "TRN2", target_bir_lowering=False, debug=False,
        num_devices=int(os.environ.get("K_NDEV", str(NCORES))),
    )

    xT_d = nc.dram_tensor("xT", [DIM, NTP], bf16, kind="ExternalInput")
    qkvw_d = nc.dram_tensor("qkv_wt", [DIM, 3 * DIM], bf16, kind="ExternalInput")
    qb_d = nc.dram_tensor("qb", [12, 64, 1], f32, kind="ExternalInput")
    projw_d = nc.dram_tensor("proj_wt", [DIM, DIM], bf16, kind="ExternalInput")
    pb_d = nc.dram_tensor("pb", [1, DIM], bf16, kind="ExternalInput")
    rpb_d = nc.dram_tensor("rpb", [2, 128, HEADS * NTOK], bf16, kind="ExternalInput")
    y_d = nc.dram_tensor("y", [NT, DIM], f32, kind="ExternalOutput")

    NTILES = [(0, 512), (512, 512), (1024, 512), (1536, 64)]
    VTILES = [(0, 512), (512, 256)]

    with tile.TileContext(nc, linearize=bool(os.environ.get("K_LINEARIZE"))) as tc:
        with (
            tc.tile_pool(name="consts", bufs=1) as consts,
            tc.tile_pool(name="acts", bufs=1) as acts,
        ):
            # ---- constants ----
            projw = consts.tile([128, 6, DIM], bf16)  # head-pair K chunks
            rpb = consts.tile([128, 2, HEADS * NTOK], bf16)
            qb = consts.tile([64, 12, 1], f32)
            pb = consts.tile([1, DIM], bf16)
            ones1 = consts.tile([1, 128], bf16)

            # persistent activations
            qkT = acts.tile([64, 2 * HEADS, NTP], bf16)  # q heads 0-11, k 12-23
            vsb = acts.tile([128, 2 * BS, HEADS, 128], bf16)  # [ones | v_h]

            xp_cm = tc.tile_pool(name="xp", bufs=1)
            xp = xp_cm.__enter__()
            xt = xp.tile([128, 6, NTP], bf16)  # x transposed [c, tok]
            vw = xp.tile([128, 6, DIM], bf16)
            wqk_cm = tc.tile_pool(name="wqk", bufs=1)
            wqk = wqk_cm.__enter__()
            qkvw = wqk.tile([128, 6, 2 * DIM], bf16)

            qkvw_v = qkvw_d[:].rearrange("(k p) n -> p k n", p=128)
            xT_v = xT_d[:].rearrange("(k p) n -> p k n", p=128)
            for k in range(6):
                nc.sync.dma_start(out=qkvw[:, k, :], in_=qkvw_v[:, k, 0 : 2 * DIM])
                nc.scalar.dma_start(out=xt[:, k, :], in_=xT_v[:, k, :])
            nc.scalar.dma_start(out=qb[:, :, :], in_=qb_d[:].rearrange("k p o -> p k o"))
            for k in range(6):
                nc.sync.dma_start(out=vw[:, k, :], in_=qkvw_v[:, k, 2 * DIM : 3 * DIM])
            for mc in range(2):
                nc.scalar.dma_start(out=rpb[:, mc, :], in_=rpb_d[mc, :, :])
            nc.scalar.dma_start(out=pb[:, :], in_=pb_d[:, :])
            projw_v = projw_d[:].rearrange("(k p) n -> p k n", p=128)
            for k in range(6):
                nc.scalar.dma_start(out=projw[:, k, :], in_=projw_v[:, k, :])
            nc.vector.memset(vsb[:, :, :, 0:64], 1.0)
            nc.vector.memset(ones1[:, :], 1.0)

            do_qkv = stop_after != "load"
            do_attn = do_qkv and stop_after != "qkv"
            do_proj = do_attn and stop_after != "attn"

            ps_v_cm = tc.tile_pool(name="ps_v", bufs=2, space="PSUM")
            ps_v = ps_v_cm.__enter__()
            ps_qk_cm = tc.tile_pool(name="ps_qk", bufs=1, space="PSUM")
            ps_qk = ps_qk_cm.__enter__()

            def emit_qk(t):
                pss = [
                    ps_qk.tile([128, 512], f32, name=f"pss{j}") for j in range(4)
                ]
                for k in range(6):
                    for j, (noff, nsz) in enumerate(NTILES):
                        nc.tensor.matmul(
                            pss[j][:, :nsz],
                            qkvw[:, k, t * 128 : (t + 1) * 128],
                            xt[:, k, noff : noff + nsz],
                            start=(k == 0),
                            stop=(k == 5),
                        )
                for j, (noff, nsz) in enumerate(NTILES):
                    if t < 6:  # q: add bias (pre-scaled on host)
                        nc.vector.tensor_scalar_add(
                            qkT[:, 2 * t, noff : noff + nsz],
                            pss[j][0:64, :nsz],
                            qb[:, 2 * t, 0:1],
                        )
                        nc.vector.tensor_scalar_add(
                            qkT[:, 2 * t + 1, noff : noff + nsz],
                            pss[j][64:128, :nsz],
                            qb[:, 2 * t + 1, 0:1],
                        )
                    else:  # k: plain copies
                        h0 = 2 * (t - 6)
                        nc.scalar.copy(
                            qkT[:, HEADS + h0, noff : noff + nsz],
                            pss[j][0:64, :nsz],
                        )
                        nc.scalar.copy(
                            qkT[:, HEADS + h0 + 1, noff : noff + nsz],
                            pss[j][64:128, :nsz],
                        )

            def emit_v(b, mc):
                msz = 128 if mc == 0 else NTOK - 128
                toff = b * NTOK + mc * 128
                psv = ps_v.tile([128, DIM], f32, name="psv")
                for k in range(6):
                    for noff, nsz in VTILES:
                        nc.tensor.matmul(
                            psv[:msz, noff : noff + nsz],
                            xt[:, k, toff : toff + msz],
                            vw[:, k, noff : noff + nsz],
                            start=(k == 0),
                            stop=(k == 5),
                        )
                nc.scalar.copy(
                    vsb[:msz, b * 2 + mc, :, 64:128],
                    psv[:msz, :].rearrange("p (h d) -> p h d", d=64),
                )

            # ---- prelude: all of q,k and v ----
            for t in range(12 if do_qkv else 0):
                emit_qk(t)
            for b in range(BS if do_qkv else 0):
                for mc in range(2):
                    emit_v(b, mc)
            ps_qk_cm.__exit__(None, None, None)
            ps_v_cm.__exit__(None, None, None)
            wqk_cm.__exit__(None, None, None)
            xp_cm.__exit__(None, None, None)

            otp_cm = tc.tile_pool(name="otp", bufs=1)
            otp = otp_cm.__enter__()
            outT = otp.tile([128, 6, NTP], bf16)  # attn out, head pairs stacked

            if stop_after == "qkv":
                nc.gpsimd.dma_start(out=y_d[0:64, :], in_=qkT[:, 0, 0:DIM])
                nc.gpsimd.dma_start(out=y_d[64:128, :], in_=qkT[:, 1, 0:DIM])
                nc.gpsimd.dma_start(
                    out=y_d[128:256, :].rearrange("p (h d) -> p h d", d=64),
                    in_=vsb[:, 0, :, 64:128],
                )
                nc.gpsimd.dma_start(out=y_d[256:320, :], in_=qkT[:, 12, 0:DIM])
                nc.gpsimd.dma_start(out=y_d[320:384, :], in_=qkT[:, 13, 0:DIM])
                nc.gpsimd.dma_start(
                    out=y_d[384:453, :].rearrange("p (h d) -> p h d", d=64),
                    in_=vsb[0:69, 1, :, 64:128],
                )

            # ---- attention interleaved with projection ----
            probes = {}
            if stop_after == "attn":
                probes["S"] = otp.tile([128, 2, NTOK], f32, name="probe_S")
                probes["e2"] = otp.tile([128, 2, NTOK], f32, name="probe_e2")
                probes["O"] = otp.tile([128, 2, NTOK], f32, name="probe_O")
                probes["Bt"] = otp.tile([64, 2, NTOK], f32, name="probe_Bt")

            e2p_cm = tc.tile_pool(name="e2p", bufs=4)
            e2p = e2p_cm.__enter__()
            bp_cm = tc.tile_pool(name="bp", bufs=2)
            bp = bp_cm.__enter__()
            ps_s_cm = tc.tile_pool(name="ps_s", bufs=2, space="PSUM")
            ps_s = ps_s_cm.__enter__()
            ps_o_cm = tc.tile_pool(name="ps_o", bufs=1, space="PSUM")
            ps_o = ps_o_cm.__enter__()
            yp_cm = tc.tile_pool(name="yp", bufs=2)
            yp = yp_cm.__enter__()
            ps_y_cm = tc.tile_pool(name="ps_y", bufs=1, space="PSUM")
            ps_y = ps_y_cm.__enter__()

            def emit_s(b, g, mc, e2):
                tb = b * NTOK
                msz = 128 if mc == 0 else NTOK - 128
                S = ps_s.tile([128, 1024], f32, name="S")
                Sv = S[:msz, :].rearrange("p (s n) -> p s n", s=2)[:, :, :NTOK]
                for hh in range(2):
                    h = 2 * g + hh
                    nc.tensor.matmul(
                        S[:msz, hh * 512 : hh * 512 + NTOK],
                        qkT[:, HEADS + h, tb + mc * 128 : tb + mc * 128 + msz],
                        qkT[:, h, tb : tb + NTOK],
                        start=True,
                        stop=True,
                    )
                nc.scalar.activation(e2[:msz, mc, :, :], Sv, EXP)
                # exp(rpb) multiplied in; split across gpsimd (large mc0
                # chunk, otherwise idle engine) and DVE (small mc1 chunk)
                eng = nc.gpsimd if mc == 0 else nc.vector
                eng.tensor_mul(
                    e2[:msz, mc, :, :],
                    e2[:msz, mc, :, :],
                    rpb[:msz, mc, 2 * g * NTOK : (2 * g + 2) * NTOK].rearrange(
                        "p (s n) -> p s n", s=2
                    ),
                )
                if probes and b == 0 and g == 0 and mc == 0:
                    nc.vector.tensor_copy(
                        probes["S"][:, :, :],
                        S[:, :].rearrange("p (s n) -> p s n", s=2)[:, :, :NTOK],
                    )
                    nc.scalar.copy(probes["e2"][:, :, :], e2[:, 0, :, :])

            def emit_o(b, g, e2, Bt):
                tb = b * NTOK
                O = ps_o.tile([128, 2, 512], f32, name="O")
                for hh in range(2):
                    h = 2 * g + hh
                    for mc in range(2):
                        msz = 128 if mc == 0 else NTOK - 128
                        nc.tensor.matmul(
                            O[:, hh, 0:NTOK],
                            vsb[:msz, b * 2 + mc, h, :],
                            e2[:msz, mc, hh, :],
                            start=(mc == 0),
                            stop=(mc == 1),
                        )
                if probes and b == 0 and g == 0:
                    nc.vector.tensor_copy(probes["O"][:, :, :], O[:, :, 0:NTOK])
                nc.vector.reciprocal_approx_fast(
                    out=Bt[:, :, :], in_=O[0:64, :, 0:NTOK]
                )
                if probes and b == 0 and g == 0:
                    nc.vector.tensor_copy(probes["Bt"][:, :, :], Bt[:, :, :])
                for hh in range(2):
                    nc.vector.tensor_mul(
                        outT[hh * 64 : (hh + 1) * 64, g, tb : tb + NTOK],
                        O[64:128, hh, 0:NTOK],
                        Bt[:, hh, :],
                    )

            def proj_steps(m):
                """Yield one proj m-tile as 8 small PE/evict steps so they can
                be woven between attention steps (keeps array duty high)."""
                moff = m * 128
                msz = min(128, NTP - moff)
                real = min(128, NT - moff)
                state = {}

                def s_pb():
                    state["Y"] = ps_y.tile([128, DIM], f32, name="Y")
                    for noff, nsz in VTILES:
                        nc.tensor.matmul(
                            state["Y"][:msz, noff : noff + nsz],
                            ones1[0:1, 0:msz],
                            pb[0:1, noff : noff + nsz],
                            start=True,
                            stop=False,
                        )

                yield s_pb
                for kp in range(6):
                    def s_kp(kp=kp):
                        for noff, nsz in VTILES:
                            nc.tensor.matmul(
                                state["Y"][:msz, noff : noff + nsz],
                                outT[:, kp, moff : moff + msz],
                                projw[:, kp, noff : noff + nsz],
                                start=False,
                                stop=(kp == 5),
                            )

                    yield s_kp

                def s_evict():
                    ysb = yp.tile([128, DIM], f32, name="ysb")
                    nc.scalar.copy(ysb[:msz, :], state["Y"][:msz, :])
                    nc.gpsimd.dma_start(
                        out=y_d[moff : moff + real, :], in_=ysb[:real, :]
                    )

                yield s_evict

            def emit_proj(m):
                for s in proj_steps(m):
                    s()

            # proj m-tile needs batches <= (128m+127)//197 fully emitted; a
            # batch's last O unit lands at position ~3 of the NEXT window.
            proj_sched = {1: [0], 2: [1], 3: [2, 3], 4: [4, 5], 5: [6],
                          6: [7, 8], 7: [9]}
            tail_proj = [10, 11, 12]

            NB = int(os.environ.get("K_NB", str(BS)))
            OLAG = int(os.environ.get("K_OLAG", "2"))
            WSTART = 6  # first attn unit that proj steps may follow: by then
            # the previous batch's last O unit (and its fmuls) are emitted
            if do_attn:
                pending = []
                for b in range(NB):
                    bigs = list(proj_sched.get(b, [])) if do_proj else []
                    units = []
                    Bt = bp.tile([64, 2, NTOK], f32, name="Bt")
                    for g in range(6):
                        e2 = e2p.tile([128, 2, 2, NTOK], bf16, name="e2")
                        units.append((emit_s, (b, g, 0, e2)))
                        units.append((emit_s, (b, g, 1, e2)))
                        pending.append((b, g, e2, Bt))
                        if len(pending) > OLAG:
                            units.append((emit_o, pending.pop(0)))
                    psteps = [s for m in bigs for s in proj_steps(m)]
                    nslot = len(units) - WSTART
                    pi = 0
                    for i, u in enumerate(units):
                        u[0](*u[1])
                        if i >= WSTART:
                            want = ((i - WSTART + 1) * len(psteps)) // max(nslot, 1)
                            while pi < want:
                                psteps[pi]()
                                pi += 1
                    while pi < len(psteps):
                        psteps[pi]()
                        pi += 1
                for p in pending:
                    emit_o(p[0], p[1], p[2], p[3])
                if do_proj:
                    for m in tail_proj:
                        emit_proj(m)

            if stop_after == "attn":
                nc.gpsimd.dma_start(out=y_d[0:128, :], in_=outT[:, 0, 0:DIM])
                for nm, rows in (("S", (128, 256)), ("e2", (256, 384)),
                                 ("O", (384, 512))):
                    nc.gpsimd.dma_start(
                        out=y_d[rows[0] : rows[1], 0 : 2 * NTOK].rearrange(
                            "p (s n) -> p s n", s=2
                        ),
                        in_=probes[nm][:, :, :],
                    )
                nc.gpsimd.dma_start(
                    out=y_d[512:576, 0 : 2 * NTOK].rearrange(
                        "p (s n) -> p s n", s=2
                    ),
                    in_=probes["Bt"][:, :, :],
                )

            for cm in (ps_y_cm, yp_cm, ps_o_cm, ps_s_cm, bp_cm, e2p_cm,
                       otp_cm):
                cm.__exit__(None, None, None)

    nc.compile()
    return nc


def _prep_inputs(x, qkv_w, q_bias, v_bias, rpb_table, proj_w, proj_b, rel_pos_index):
    bf16 = ml_dtypes.bfloat16
    x = np.asarray(x, np.float32)
    qkv_w = np.asarray(qkv_w, np.float32)
    q_bias = np.asarray(q_bias, np.float32)
    v_bias = np.asarray(v_bias, np.float32)
    rpb_table = np.asarray(rpb_table, np.float32)
    proj_w = np.asarray(proj_w, np.float32)
    proj_b = np.asarray(proj_b, np.float32)
    rel_pos_index = np.asarray(rel_pos_index)

    qkv_wt = qkv_w.T.copy()  # [768, 2304]
    qkv_wt[:, :DIM] *= SCALE
    qkv_wt = np.ascontiguousarray(qkv_wt, dtype=bf16)

    qb = (q_bias * SCALE).reshape(12, 64, 1).astype(np.float32)

    proj_wt = np.ascontiguousarray(proj_w.T, dtype=bf16)
    pb_eff = (proj_b + proj_w @ v_bias).reshape(1, DIM).astype(bf16)

    # bias[h, n, m] = rpb_table[rel_pos_index[n, m], h]; store exp()
    # (multiplicative) as [m-chunk, m-in-chunk, h*197 + n]
    bias_nmh = rpb_table[rel_pos_index]  # [n, m, h]
    er = np.exp(bias_nmh.transpose(1, 2, 0))  # [m, h, n]
    er = er.reshape(NTOK, HEADS * NTOK)
    er_pad = np.ones((256, HEADS * NTOK), np.float32)
    er_pad[:NTOK] = er
    rpb = np.ascontiguousarray(er_pad.reshape(2, 128, HEADS * NTOK), dtype=bf16)

    shared = {
        "qkv_wt": qkv_wt,
        "qb": qb,
        "proj_wt": proj_wt,
        "pb": pb_eff,
        "rpb": rpb,
    }
    in_maps = []
    for c in range(NCORES):
        xc = x[c * BS : (c + 1) * BS].reshape(NT, DIM)
        xp = np.zeros((NTP, DIM), np.float32)
        xp[:NT] = xc
        xT = np.ascontiguousarray(xp.T, dtype=bf16)  # [768, 1600]
        in_maps.append({"xT": xT, **shared})
    return in_maps


def run(inputs, trace=False):
    """Build (cached), run on 8 cores, return (y_full, BassKernelResults)."""
    from concourse.bass_utils import run_bass_kernel_spmd

    if "nc" not in _CACHE:
        _CACHE["nc"] = _build_bass()
    nc = _CACHE["nc"]
    in_maps = _prep_inputs(**{k: inputs[k] for k in (
        "x", "qkv_w", "q_bias", "v_bias", "rpb_table", "proj_w", "proj_b",
        "rel_pos_index")})
    try:
        res = run_bass_kernel_spmd(
            nc, in_maps, core_ids=list(range(NCORES)), trace=trace
        )
    except ModuleNotFoundError:
        # NTFF profile hook unavailable in this container; run untraced
        res = run_bass_kernel_spmd(
            nc, in_maps, core_ids=list(range(NCORES)), trace=False
        )
    y = np.concatenate(
        [res.results[c]["y"].reshape(BS, NTOK, DIM) for c in range(NCORES)], axis=0
    )
    return np.ascontiguousarray(y, np.float32), res


def kernel(**inputs) -> np.ndarray:
    y, _ = run(inputs, trace=False)
    return y
